# revision 20
# baseline (speedup 1.0000x reference)
"""EGNN (2-layer, graph pooling) Trainium2 SPMD kernel over 8 NeuronCores.

v2: edges dst-sorted and sharded by destination-node range. Per 128-node
dst tile the kernel batch-gathers BOTH endpoint projections via indirect
DMA (src from a full projected table, dst from the core-local table),
builds the edge MLP in h'-major form on the TensorEngine (weights
stationary; per-block identity-matmuls transpose the gathered sums into
the PSUM accumulation), generates the scatter one-hot on-chip from local
dst indices, and scatter-adds with one matmul per 128-edge block. Matmul
operands are bf16 (fp32 PSUM accumulation). The layer-1 feature table is
exchanged with an AllGather; graph pooling uses one-hot matmuls and a
final AllReduce. Walrus in this environment accepts one sync-wait per
instruction, so a JSON-level pass splits multi-wait instructions onto
NoOp carriers.
"""
import sys
sys.path.insert(0, '/opt/trn_rl_repo')
import concourse.tile as tile_mod
from concourse.vector_clock import ScopedClock


def _patched_drain_and_barrier(self, tick_clock, wait_clock):
    nc = self.nc
    probe = nc.sync.nop(nofuse=True)
    wait_clock.add_sem_waits(probe.ins, ScopedClock({None: tick_clock.global_clock}))
    waits = list(probe.ins.sync_info.on_wait)
    probe.ins.sync_info.on_wait = []
    import concourse.mybir as mybir
    for w in waits:
        carrier = nc.sync.nop(nofuse=True)
        if carrier.ins.sync_info is None:
            carrier.ins.sync_info = mybir.SyncInfo(on_wait=[], on_update=[])
        carrier.ins.sync_info.on_wait = [w]
    nc.sync.drain()

    nc.all_engine_barrier()
    assert self.sems is not None
    popped = nc._tile_sem_poison_stack.pop()
    assert popped is self._sem_poison
    nc.clear_and_free_semaphores(list(self.sems.allocated().values()))
    nc.all_engine_barrier()


def apply_patch():
    tile_mod.TileContext._drain_and_barrier = _patched_drain_and_barrier


def _legalize_waits_json(mod: dict) -> dict:
    """Walrus in this env accepts at most ONE sync wait per instruction.
    Split extra waits onto same-engine NoOp carriers inserted just before."""
    n_new = [0]
    for fn in mod.get('functions', []):
        for blk in fn.get('blocks', []):
            insts = blk.get('instructions', [])
            out = []
            for inst in insts:
                si = inst.get('sync_info') or {}
                waits = si.get('on_wait') or []
                if len(waits) > 1:
                    eng = inst.get('engine')
                    for w in waits[:-1]:
                        n_new[0] += 1
                        out.append({
                            'debug': inst.get('debug', 0),
                            'engine': eng, 'ins': [], 'outs': [],
                            'name': 'I-waitfix-%d' % n_new[0],
                            'opcode': 'NoOp',
                            'sync_info': {'on_update': [], 'on_wait': [w]},
                        })
                    si['on_wait'] = [waits[-1]]
                out.append(inst)
            blk['instructions'] = out
    return mod


def apply_json_patch():
    import orjson
    import concourse.bass as bass_mod
    orig = bass_mod.Bass.to_json_bytes
    def to_json_bytes(self):
        raw = orig(self)
        mod = orjson.loads(raw)
        mod = _legalize_waits_json(mod)
        return orjson.dumps(mod)
    bass_mod.Bass.to_json_bytes = to_json_bytes


import math
import numpy as np
import ml_dtypes
import concourse.bass as bass
import concourse.mybir as mybir
from concourse.tile import TileContext
from concourse import bass_utils
from concourse.masks import make_identity
from concourse import library_config
apply_patch(); apply_json_patch()

f32 = mybir.dt.float32
bf16 = mybir.dt.bfloat16
i32 = mybir.dt.int32
AF = mybir.ActivationFunctionType
ALU = mybir.AluOpType
AX = mybir.AxisListType
P = 128
NC = 8
GRP = 4          # blocks per h PSUM group (GRP*128 f32 = one 2KB bank)
DBG = False

bft = ml_dtypes.bfloat16


def host_prep(inputs, n_tiles_per_core):
    SH = n_tiles_per_core * P
    NPAD = SH * NC
    T = n_tiles_per_core
    N = inputs['x'].shape[0]
    src = np.asarray(inputs['edge_index'][0], np.int64)
    dst = np.asarray(inputs['edge_index'][1], np.int64)
    ea = np.asarray(inputs['edge_attr'], np.float32)
    order = np.argsort(dst, kind='stable')
    src, dst, ea = src[order], dst[order], ea[order]
    core_of = dst // SH
    tile_of = (dst % SH) // P

    counts = np.zeros((NC, T), np.int64)
    for c in range(NC):
        m = core_of == c
        tl, cn = np.unique(tile_of[m], return_counts=True)
        counts[c, tl] = cn
    K = np.maximum(1, np.ceil(counts / P).astype(np.int64).max(axis=0))
    offs = np.concatenate([[0], np.cumsum(K)]).astype(np.int64)
    TOT = int(offs[-1])

    src_pm = np.zeros((NC, P, TOT), np.int32)          # global src node id
    dstl_pm = np.zeros((NC, P, TOT), np.int32)         # dst local to core
    nloc_t = np.full((NC, P, TOT), -1.0, np.float32)   # dst local to tile, pad -1
    src_w = np.zeros((NC, P, TOT * 8), np.int16)       # dma_gather 16-p wrap
    dstl_w = np.zeros((NC, P, TOT * 8), np.int16)
    ea_t = np.zeros((NC, 16, TOT * P), bft)
    for c in range(NC):
        m = core_of == c
        s_c, d_c, e_c, t_c = src[m], dst[m], ea[m], tile_of[m]
        for t in range(T):
            mt = t_c == t
            sc, dc, ec = s_c[mt], d_c[mt], e_c[mt]
            dloc_core = (dc - c * SH).astype(np.int64)
            nloc = dloc_core - t * P
            n_e = len(sc)
            for k in range(int(K[t])):
                blk = int(offs[t]) + k
                lo, hi = k * P, min((k + 1) * P, n_e)
                cnt = max(0, hi - lo)
                if cnt > 0:
                    src_pm[c, :cnt, blk] = sc[lo:hi]
                    dstl_pm[c, :cnt, blk] = dloc_core[lo:hi]
                    nloc_t[c, :cnt, blk] = nloc[lo:hi]
                    ea_t[c, :, blk * P: blk * P + cnt] = ec[lo:hi].T.astype(bft)
    for c in range(NC):
        for t in range(T):
            off, Kt = int(offs[t]), int(K[t])
            lin_s = src_pm[c][:, off:off + Kt].T.ravel()     # i = k*128+p
            lin_d = dstl_pm[c][:, off:off + Kt].T.ravel()
            bs = lin_s.reshape(Kt * 8, 16).T.astype(np.int16)
            bd = lin_d.reshape(Kt * 8, 16).T.astype(np.int16)
            for c8 in range(8):      # replicated per Q7 core stripe
                src_w[c][c8 * 16:(c8 + 1) * 16, off * 8:(off + Kt) * 8] = bs
                dstl_w[c][c8 * 16:(c8 + 1) * 16, off * 8:(off + Kt) * 8] = bd
    # pooling one-hots + counts
    batch = np.asarray(inputs['batch'], np.int64)
    G_ = 64
    bpool = np.zeros((NC, P, T * 64), bft)
    for c in range(NC):
        base = c * SH
        for t in range(T):
            n0 = base + t * P
            n1 = min(n0 + P, N)
            if n1 > n0:
                rows = np.arange(n1 - n0)
                bpool[c, rows, t * 64 + batch[n0:n1]] = 1.0
    cnts = np.bincount(batch, minlength=G_).astype(np.float32)
    invcnt = (1.0 / np.maximum(cnts, 1.0)).reshape(G_, 1)

    # node data
    xf = np.zeros((NPAD, P), np.float32)
    xf[:N] = np.asarray(inputs['x'], np.float32)
    xT = np.ascontiguousarray(xf.T.astype(bft))            # (128, NPAD)
    pf = np.zeros((NPAD, 4), np.float32)
    pf[:N, 0:3] = np.asarray(inputs['pos'], np.float32)
    # tiled pos: (P, nt*4) with [p, 4t:4t+4] = pos[t*128+p]
    NT_FULL = NPAD // P
    pos_tf = np.ascontiguousarray(
        pf.reshape(NT_FULL, P, 4).transpose(1, 0, 2).reshape(P, NT_FULL * 4))
    iotab = np.tile(np.arange(P, dtype=np.float32), (P, 1))
    return dict(SH=SH, NPAD=NPAD, T=T, K=[int(k) for k in K],
                offs=[int(o) for o in offs], TOT=TOT, src_pm=src_pm,
                dstl_pm=dstl_pm, src_w=src_w, dstl_w=dstl_w,
                nloc_t=nloc_t, ea_t=ea_t, bpool=bpool,
                invcnt=invcnt, xT=xT, pos_tf=pos_tf, iotab=iotab)


def host_weights(inputs):
    w = {}
    for L in range(2):
        mw = np.asarray(inputs[f'l{L}_mlp_w'], np.float32)
        w[f'wproj{L}'] = np.concatenate([mw[0:128], mw[128:256]],
                                        axis=1).astype(bft)     # [dst|src]
        w[f'wea{L}'] = np.ascontiguousarray(mw[256:272]).astype(bft)
        w[f'wrrep{L}'] = np.tile(mw[272:273], (P, 1)).astype(bft)
        ew = np.asarray(inputs[f'l{L}_edge_w'], np.float32)
        eb = np.asarray(inputs[f'l{L}_edge_b'], np.float32)
        we1s = np.zeros((65, 64), np.float32)
        we1s[0:64] = ew; we1s[64] = eb
        w[f'we1s{L}'] = we1s.astype(bft)
        cw = np.asarray(inputs[f'l{L}_coord_w'], np.float32)
        w[f'cwrep{L}'] = np.tile(cw[:, 0][None, :], (P, 1)).astype(bft)
        w[f'cb{L}'] = float(np.asarray(inputs[f'l{L}_coord_b'], np.float32)[0])
        n1 = np.asarray(inputs[f'l{L}_node_w1'], np.float32)
        w[f'wn1x{L}'] = np.ascontiguousarray(n1[0:128]).astype(bft)
        w[f'wn1a{L}'] = np.ascontiguousarray(n1[128:192]).astype(bft)
        w[f'nb1_{L}'] = np.asarray(
            inputs[f'l{L}_node_b1'], np.float32).reshape(64, 1)  # bias col
        w[f'wn2b{L}'] = np.concatenate(
            [np.asarray(inputs[f'l{L}_node_w2'], np.float32),
             np.asarray(inputs[f'l{L}_node_b2'], np.float32)[None, :]],
            0).astype(bft)
    w['wo1'] = np.asarray(inputs['out_w1'], np.float32).astype(bft)
    w['wo1b'] = np.asarray(inputs['out_b1'], np.float32).reshape(P, 1)
    w['wo2'] = np.asarray(inputs['out_w2'], np.float32).astype(bft)
    w['wo2b'] = np.asarray(inputs['out_b2'], np.float32).reshape(32, 1)
    return w


# name -> (shape, dtype); cb0/cb1 are python floats, not tensors
WSPEC = dict(wproj0=((P, P), bf16), wproj1=((P, P), bf16),
             wea0=((16, 64), bf16), wea1=((16, 64), bf16),
             wrrep0=((P, 64), bf16), wrrep1=((P, 64), bf16),
             we1s0=((65, 64), bf16), we1s1=((65, 64), bf16),
             cwrep0=((P, 64), bf16), cwrep1=((P, 64), bf16),
             wn1x0=((P, 64), bf16), wn1x1=((P, 64), bf16),
             wn1a0=((64, 64), bf16), wn1a1=((64, 64), bf16),
             nb1_0=((64, 1), f32), nb1_1=((64, 1), f32),
             wn2b0=((65, P), bf16), wn2b1=((65, P), bf16),
             wo1=((P, P), bf16), wo1b=((P, 1), f32),
             wo2=((P, 32), bf16), wo2b=((32, 1), f32))


def _bc_k(ap2d, Kt):
    """(P, C) -> (P, Kt, C) broadcast view (k stride 0)."""
    a = ap2d.rearrange("p (k c) -> p k c", k=1)
    new = [list(d) for d in a.ap]
    new[1] = [0, Kt]
    return bass.AP(a.tensor, a.offset, new)


def _bc_c(ap2d, C):
    """(P, Kt) -> (P, Kt, C) broadcast view (c stride 0)."""
    a = ap2d.rearrange("p (k o) -> p k o", o=1)
    new = [list(d) for d in a.ap]
    new[2] = [0, C]
    return bass.AP(a.tensor, a.offset, new)


def build(st, cb):
    SH, NPAD, T, K, offs, TOT = (st['SH'], st['NPAD'], st['T'], st['K'],
                                 st['offs'], st['TOT'])
    NT_FULL = NPAD // P
    G_, OUT = 64, 32
    KMAX = max(K)

    nc = bass.Bass("TRN2")
    dram = {}
    def din(name, shape, dt=f32):
        dram[name] = nc.dram_tensor(name, shape, dt, kind="ExternalInput")
        return dram[name]

    xT_full = din('xT_full', (P, NPAD), bf16)
    xT_own = din('xT_own', (P, SH), bf16)
    pos_tf_d = din('pos_tf', (P, NT_FULL * 4))
    pos_own_d = din('pos_own', (P, T * 4))
    i16 = mybir.dt.int16
    srcw_d = din('src_w', (P, TOT * 8), i16)
    dstlw_d = din('dstl_w', (P, TOT * 8), i16)
    nloc_d = din('nloc_t', (P, TOT))
    ea_td = din('ea_t', (16, TOT * P), bf16)
    bpool_d = din('bpool', (P, T * 64), bf16)
    invcnt_d = din('invcnt', (G_, 1))
    iotab_d = din('iotab', (P, P))
    for n, (shp, dt) in WSPEC.items():
        din(n, shp, dt)
    out_ext = nc.dram_tensor('out', (G_, OUT), f32, kind="ExternalOutput")

    ts0 = nc.dram_tensor('ts0', (NPAD, P), bf16, kind='ExternalOutput')
    ts1sh = nc.dram_tensor('ts1sh', (SH, P), bf16)
    ts1 = nc.dram_tensor('ts1', (NPAD, P), bf16, addr_space="Shared")
    xdp = [nc.dram_tensor('xdp0', (SH, P), bf16, kind='ExternalOutput'),
           nc.dram_tensor('xdp1', (SH, P), bf16)]
    if DBG:
        dbg1 = nc.dram_tensor('dbg1', (P, 68 * 4), bf16, kind='ExternalOutput')
        dbg2 = nc.dram_tensor('dbg2', (P, 64 + P + 64), bf16,
                              kind='ExternalOutput')
        dbg3 = nc.dram_tensor('dbg3', (65, 17 * P), bf16, kind='ExternalOutput')
        dbg5 = nc.dram_tensor('dbg5', (P, 64), bf16, kind='ExternalOutput')
        dbg6 = nc.dram_tensor('dbg6', (P, 4), f32, kind='ExternalOutput')
        dbg7 = nc.dram_tensor('dbg7', (P, P), bf16, kind='ExternalOutput')
        dbg8 = nc.dram_tensor('dbg8', (P, 68), bf16, kind='ExternalOutput')
        dbg9 = nc.dram_tensor('dbg9', (64, P), f32, kind='ExternalOutput')
        dbg10 = nc.dram_tensor('dbg10', (P, 17 * 68), bf16,
                               kind='ExternalOutput')
        dbg11 = nc.dram_tensor('dbg11', (P, 17 * P), bf16,
                               kind='ExternalOutput')
        dbg12 = nc.dram_tensor('dbg12', (P, 17 * 64), bf16,
                               kind='ExternalOutput')
        dbg13 = nc.dram_tensor('dbg13', (P, 17 * 68), bf16,
                               kind='ExternalOutput')
        dbg14 = nc.dram_tensor('dbg14', (P, 17), f32, kind='ExternalOutput')
        dbg15 = nc.dram_tensor('dbg15', (P, 17 * 68), bf16,
                               kind='ExternalOutput')
    gs_in = nc.dram_tensor('gs_in', (G_, P), f32)
    gs_out = nc.dram_tensor('gs_out', (G_, P), f32, addr_space="Shared")

    with TileContext(nc) as tc:
        with (tc.tile_pool(name="pers", bufs=1) as pers,
              tc.tile_pool(name="sa", bufs=3) as sa,
              tc.tile_pool(name="sb", bufs=2) as sb,
              tc.tile_pool(name="sg", bufs=2) as sgp,
              tc.tile_pool(name="ph", bufs=2, space="PSUM") as ps_h,
              tc.tile_pool(name="pe1", bufs=2, space="PSUM") as ps_e1,
              tc.tile_pool(name="pagg", bufs=1, space="PSUM") as ps_agg,
              tc.tile_pool(name="pnd", bufs=2, space="PSUM") as ps_nd,
              tc.tile_pool(name="ppool", bufs=1, space="PSUM") as ps_pool):

            nc.gpsimd.load_library(library_config.mlp)
            identb = pers.tile([P, P], bf16, name="identb", tag="identb")
            make_identity(nc, identb[:])
            iota_t = pers.tile([P, P], f32, name="iota", tag="iota")
            nc.sync.dma_start(iota_t[:], iotab_d[:, :])
            invcnt_t = pers.tile([G_, 1], f32, name="invc", tag="invc")
            nc.sync.dma_start(invcnt_t[:], invcnt_d[:, :])
            W = {}
            for n, (shp, dt) in WSPEC.items():
                W[n] = pers.tile(list(shp), dt, name="w_" + n, tag="w_" + n)
                nc.sync.dma_start(W[n][:], dram[n][:, :])
            xT_a = pers.tile([P, SH], bf16, name="xT_a", tag="xT_a")
            nc.sync.dma_start(xT_a[:], xT_own[:, :])
            xT_b = pers.tile([P, SH], bf16, name="xT_b", tag="xT_b")
            pos_own = pers.tile([P, T * 4], f32, name="pos_own", tag="pos_own")
            nc.sync.dma_start(pos_own[:], pos_own_d[:, :])
            posn_all = pers.tile([P, T * 4], f32, name="posn_all", tag="posn_all")
            pos_tfull = pers.tile([P, NT_FULL * 4], f32, name="pos_tf",
                                  tag="pos_tf")
            nc.sync.dma_start(pos_tfull[:], pos_tf_d[:, :])
            bpool_t = pers.tile([P, T * 64], bf16, name="bpool", tag="bpool")
            nc.sync.dma_start(bpool_t[:], bpool_d[:, :])
            srcw_t = pers.tile([P, TOT * 8], i16, name="srcw", tag="srcw")
            nc.sync.dma_start(srcw_t[:], srcw_d[:, :])
            dstlw_t = pers.tile([P, TOT * 8], i16, name="dstlw", tag="dstlw")
            nc.sync.dma_start(dstlw_t[:], dstlw_d[:, :])
            nloc_res = pers.tile([P, TOT], f32, name="nlocr", tag="nlocr")
            nc.sync.dma_start(nloc_res[:], nloc_d[:, :])
            # persistent double-buffered tiles with constant regions
            hTs_b = [pers.tile([65, KMAX * P], bf16, name=f"hTs{i}", tag=f"hTs{i}")
                     for i in range(2)]
            scat_b = [pers.tile([P, KMAX * 68], bf16, name=f"scat{i}",
                                tag=f"scat{i}") for i in range(2)]
            zst_b = [pers.tile([65, P], bf16, name=f"zst{i}", tag=f"zst{i}")
                     for i in range(2)]
            for i in range(2):
                nc.vector.memset(hTs_b[i][64:65, :], 1.0)
                sc3i = scat_b[i][:].rearrange("p (k c) -> p k c", c=68)
                nc.vector.memset(sc3i[:, :, 64:65], 1.0)
                nc.vector.memset(zst_b[i][64:65, :], 1.0)

            # ---------------- stage A: full-N ts0 table ----------------
            for ti in range(NT_FULL):
                xt = sa.tile([P, P], bf16, name="ax", tag="ax")
                nc.gpsimd.dma_start(xt[:], xT_full[:, ti * P:(ti + 1) * P])
                nb_ = ps_nd.tile([P, 512], f32, name="pnd", tag="pnd",
                                 space="PSUM")
                nc.tensor.matmul(out=nb_[:, 0:64], lhsT=xt[:],
                                 rhs=W['wproj0'][:, 64:128],
                                 start=True, stop=True, skip_group_check=True)
                tst = sa.tile([P, 68], bf16, name="tst", tag="tst")
                nc.scalar.activation(out=tst[:, 0:64], in_=nb_[:, 0:64],
                                     func=AF.Copy)
                nc.vector.tensor_scalar_mul(
                    tst[:, 64:68], pos_tfull[:, ti * 4:(ti + 1) * 4], -1.0)
                nc.gpsimd.dma_start(ts0[ti * P:(ti + 1) * P, 0:68], tst[:])

            # ---------------- stage A-own: xdp0 ------------------------
            for t in range(T):
                nb_ = ps_nd.tile([P, 512], f32, name="pnd", tag="pnd",
                                 space="PSUM")
                nc.tensor.matmul(out=nb_[:, 0:64], lhsT=xT_a[:, t * P:(t + 1) * P],
                                 rhs=W['wproj0'][:, 0:64], start=True, stop=True,
                                 skip_group_check=True)
                xds = sa.tile([P, 68], bf16, name="xds", tag="xds")
                nc.scalar.activation(out=xds[:, 0:64], in_=nb_[:, 0:64],
                                     func=AF.Copy)
                nc.vector.tensor_copy(out=xds[:, 64:68],
                                      in_=pos_own[:, t * 4:(t + 1) * 4])
                nc.gpsimd.dma_start(xdp[0][t * P:(t + 1) * P, 0:68], xds[:])

            nidx_regs = {}
            def nidx_reg(n):
                if n not in nidx_regs:
                    nidx_regs[n] = nc.gpsimd.to_reg(n)
                return nidx_regs[n]

            # ---------------- edge + node stage, per layer --------------
            def layer(L, ts_dram, xdp_dram, pos_tile_of, xin_T, xout_T):
                sfx = str(L)
                for t in range(T):
                    Kt = K[t]
                    off = offs[t]
                    ng = (Kt + GRP - 1) // GRP
                    buf = (L * T + t) % 2
                    hT_s = hTs_b[buf]
                    scat = scat_b[buf]
                    zst = zst_b[buf]
                    sc3 = scat[:].rearrange("p (k c) -> p k c", c=68)

                    nloc_tt = nloc_res[:, off:off + Kt]
                    eat = sgp.tile([16, KMAX * P], bf16, name="eat", tag="eat")
                    nc.gpsimd.dma_start(eat[:, 0:Kt * P],
                                        ea_td[:, off * P:(off + Kt) * P])
                    g_src = sgp.tile([P, KMAX * P], bf16, name="gsrc", tag="gsrc")
                    gs3 = g_src[:].rearrange("p (k c) -> p k c", c=P)
                    g_dst = sgp.tile([P, KMAX * P], bf16, name="gdst", tag="gdst")
                    gd3 = g_dst[:].rearrange("p (k c) -> p k c", c=P)
                    for g3v, tbl, idxt in ((gs3, ts_dram, srcw_t),
                                           (gd3, xdp_dram, dstlw_t)):
                        k0 = 0
                        while k0 < Kt:
                            kn = min(8, Kt - k0)      # <=1024 descs per ring
                            nc.gpsimd.dma_gather(
                                out_ap=g3v[:, k0:k0 + kn, :], in_ap=tbl[:, :],
                                idxs_ap=idxt[:, (off + k0) * 8:(off + k0 + kn) * 8],
                                num_idxs=kn * P, num_idxs_reg=nidx_reg(kn * P),
                                elem_size=P)
                            k0 += kn

                    gsum = sb.tile([P, KMAX * 68], bf16, name="gsum", tag="gsum")
                    g3s = gsum[:].rearrange("p (k c) -> p k c", c=68)
                    nc.vector.tensor_tensor(out=g3s[:, 0:Kt, :],
                                            in0=gs3[:, 0:Kt, 0:68],
                                            in1=gd3[:, 0:Kt, 0:68], op=ALU.add)
                    g3 = gsum[:].rearrange("p (k c) -> p k c", c=68)
                    # radial = sum(diff^2), diff = gsum[:, k, 64:67]
                    dsq = sb.tile([P, KMAX * 3], f32, name="dsq", tag="dsq")
                    q3 = dsq[:].rearrange("p (k c) -> p k c", c=3)
                    nc.vector.tensor_tensor(out=q3[:, 0:Kt, :],
                                            in0=g3[:, 0:Kt, 64:67],
                                            in1=g3[:, 0:Kt, 64:67], op=ALU.mult)
                    radst = sb.tile([P, KMAX], f32, name="rad", tag="rad")
                    nc.vector.tensor_reduce(
                        out=radst[:, 0:Kt].rearrange("p (k o) -> p k o", o=1),
                        in_=q3[:, 0:Kt, :], axis=AX.X, op=ALU.add)
                    # gsum2 = gsum[:, k, 0:64] + radial * w_r  (fused per block)
                    gsum2 = sb.tile([P, KMAX * 64], bf16, name="gsum2",
                                    tag="gsum2")
                    for k in range(Kt):
                        nc.vector.scalar_tensor_tensor(
                            out=gsum2[:, k * 64:(k + 1) * 64],
                            in0=W['wrrep' + sfx][:],
                            scalar=radst[:, k:k + 1],
                            in1=gsum[:, k * 68:k * 68 + 64],
                            op0=ALU.mult, op1=ALU.add)
                    # scatter one-hot from nloc (split gpsimd / DVE)
                    s4 = sb.tile([P, KMAX * P], bf16, name="s4", tag="s4")
                    for k in range(Kt):
                        eng = nc.gpsimd if k % 2 == 0 else nc.vector
                        eng.tensor_scalar(
                            out=s4[:, k * P:(k + 1) * P], in0=iota_t[:],
                            scalar1=nloc_tt[:, k:k + 1] if hasattr(nloc_tt, 'tensor') else nloc_tt, scalar2=None,
                            op0=ALU.is_equal)
                    # h'-major h: psum groups of GRP blocks
                    for g in range(ng):
                        nb = min(GRP, Kt - g * GRP)
                        hp = ps_h.tile([64, GRP * P], f32, name="hp", tag="hp",
                                       space="PSUM")
                        for j in range(nb):
                            k = g * GRP + j
                            nc.tensor.matmul(
                                out=hp[:, j * P:(j + 1) * P],
                                lhsT=gsum2[:, k * 64:(k + 1) * 64],
                                rhs=identb[:],
                                start=True, stop=False)
                            nc.tensor.matmul(
                                out=hp[:, j * P:(j + 1) * P],
                                lhsT=W['wea' + sfx][:],
                                rhs=eat[:, k * P:(k + 1) * P],
                                start=False, stop=True)
                        nc.scalar.activation(
                            out=hT_s[0:64, g * GRP * P:(g * GRP + nb) * P],
                            in_=hp[:, 0:nb * P], func=AF.Silu)
                    # e1 = silu(hT @ we1s) edge-major, into scat[:, :, 0:64]
                    for g in range(ng):
                        nb = min(GRP, Kt - g * GRP)
                        ep = ps_e1.tile([P, GRP * 64], f32, name="ep", tag="ep",
                                        space="PSUM")
                        for j in range(nb):
                            k = g * GRP + j
                            nc.tensor.matmul(out=ep[:, j * 64:(j + 1) * 64],
                                             lhsT=hT_s[:, k * P:(k + 1) * P],
                                             rhs=W['we1s' + sfx][:],
                                             start=True, stop=True)
                        e3 = ep[:, 0:nb * 64].rearrange("p (k c) -> p k c", c=64)
                        nc.scalar.activation(
                            out=sc3[:, g * GRP:g * GRP + nb, 0:64],
                            in_=e3[:, :, :], func=AF.Silu)
                    if L == 0:
                        # sgate = silu(e1 @ cw + cb); coord cols
                        e1m = sb.tile([P, KMAX * 64], bf16, name="e1m",
                                      tag="e1m")
                        m3 = e1m[:].rearrange("p (k c) -> p k c", c=64)
                        nc.vector.tensor_tensor(
                            out=m3[:, 0:Kt, :], in0=sc3[:, 0:Kt, 0:64],
                            in1=_bc_k(W['cwrep' + sfx][:], Kt), op=ALU.mult)
                        sgt = sb.tile([P, KMAX], f32, name="sgt", tag="sgt")
                        nc.vector.tensor_reduce(
                            out=sgt[:, 0:Kt].rearrange("p (k o) -> p k o", o=1),
                            in_=m3[:, 0:Kt, :], axis=AX.X, op=ALU.add)
                        sgs = sb.tile([P, KMAX], bf16, name="sgs", tag="sgs")
                        nc.scalar.activation(out=sgs[:, 0:Kt], in_=sgt[:, 0:Kt],
                                             func=AF.Silu, bias=cb[L])
                        nc.vector.tensor_tensor(out=sc3[:, 0:Kt, 65:68],
                                                in0=g3[:, 0:Kt, 64:67],
                                                in1=_bc_c(sgs[:, 0:Kt], 3),
                                                op=ALU.mult)
                    # scatter-add into node aggregate
                    NAGG = 68 if L == 0 else 65
                    pagg = ps_agg.tile([P, 68], f32, name="pagg", tag="pagg",
                                       space="PSUM")
                    for k in range(Kt):
                        nc.tensor.matmul(out=pagg[:, 0:NAGG],
                                         lhsT=s4[:, k * P:(k + 1) * P],
                                         rhs=scat[:, k * 68:k * 68 + NAGG],
                                         start=(k == 0), stop=(k == Kt - 1))
                    if DBG and L == 0 and t == 0:
                        nc.sync.dma_start(dbg10[:, 0:Kt * 68],
                                          scat[:, 0:Kt * 68])
                        nc.sync.dma_start(dbg11[:, 0:Kt * P], s4[:, 0:Kt * P])
                        nc.sync.dma_start(dbg12[:, 0:Kt * 64],
                                          gsum2[:, 0:Kt * 64])
                        nc.sync.dma_start(dbg13[:, 0:Kt * 68],
                                          gsum[:, 0:Kt * 68])
                        nc.sync.dma_start(dbg14[:, 0:Kt], radst[:, 0:Kt])
                        nc.sync.dma_start(dbg1[:, 0:68], g_src[:, 0:68])
                        nc.sync.dma_start(dbg1[:, 68:136], g_dst[:, 0:68])
                        nc.sync.dma_start(dbg15[:, 0:Kt * 68],
                                          gs3[:, 0:Kt, 0:68])
                        nc.sync.dma_start(dbg1[:, 136:204], gsum[:, 0:68])
                        nc.sync.dma_start(dbg1[:, 204:272], scat[:, 0:68])
                        nc.sync.dma_start(dbg2[:, 0:64], gsum2[:, 0:64])
                        nc.sync.dma_start(dbg2[:, 64:64 + P], s4[:, 0:P])
                        nc.sync.dma_start(dbg2[:, 64 + P:64 + P + 64],
                                          e1m[:, 0:64])
                        nc.sync.dma_start(dbg3[:, :], hT_s[:, 0:17 * P])
                    # ---- node stage ----
                    eagg = sb.tile([P, 64], bf16, name="eagg", tag="eagg")
                    nc.scalar.activation(out=eagg[:], in_=pagg[:, 0:64],
                                         func=AF.Copy)
                    if L == 0:
                        deg1 = sb.tile([P, 1], f32, name="deg", tag="deg")
                        nc.vector.tensor_scalar_max(deg1[:], pagg[:, 64:65], 1.0)
                        inv = sb.tile([P, 1], f32, name="inv", tag="inv")
                        nc.vector.reciprocal(out=inv[:], in_=deg1[:])
                        posn = sb.tile([P, 4], f32, name="posn", tag="posn")
                        nc.vector.tensor_scalar_mul(posn[:, 0:3], pagg[:, 65:68],
                                                    inv[:, 0:1])
                        nc.vector.tensor_tensor(
                            out=posn[:, 0:3], in0=posn[:, 0:3],
                            in1=pos_tile_of[:, t * 4:t * 4 + 3], op=ALU.add)
                        nc.vector.memset(posn[:, 3:4], 0.0)
                    nb_ = ps_nd.tile([P, 512], f32, name="pnd", tag="pnd",
                                     space="PSUM")
                    nc.tensor.matmul(out=nb_[0:64, 0:P], lhsT=eagg[:],
                                     rhs=identb[:], start=True, stop=True,
                                     skip_group_check=True)
                    eaT = sb.tile([64, P], bf16, name="eaT", tag="eaT")
                    nc.scalar.activation(out=eaT[:], in_=nb_[0:64, 0:P],
                                         func=AF.Copy)
                    nc.tensor.matmul(out=nb_[0:64, P:2 * P],
                                     lhsT=W['wn1x' + sfx][:],
                                     rhs=xin_T[:, t * P:(t + 1) * P],
                                     start=True, stop=False,
                                     skip_group_check=True)
                    nc.tensor.matmul(out=nb_[0:64, P:2 * P],
                                     lhsT=W['wn1a' + sfx][:],
                                     rhs=eaT[:], start=False, stop=True,
                                     skip_group_check=True)
                    nc.scalar.activation(out=zst[0:64, :], in_=nb_[0:64, P:2 * P],
                                         func=AF.Silu,
                                         bias=W['nb1_' + sfx][:, 0:1])
                    nc.tensor.matmul(out=nb_[:, 2 * P:3 * P],
                                     lhsT=W['wn2b' + sfx][:],
                                     rhs=zst[:], start=True, stop=True,
                                     skip_group_check=True)
                    nc.scalar.activation(out=xout_T[:, t * P:(t + 1) * P],
                                         in_=nb_[:, 2 * P:3 * P], func=AF.Copy)
                    if DBG and L == 0 and t == 0:
                        nc.sync.dma_start(dbg5[:, :], eagg[:])
                        nc.sync.dma_start(dbg6[:, :], posn[:])
                        nc.sync.dma_start(dbg7[:, :],
                                          xout_T[:, t * P:(t + 1) * P])
                    if L == 0:
                        nc.tensor.matmul(out=nb_[:, 3 * P:3 * P + 64],
                                         lhsT=xout_T[:, t * P:(t + 1) * P],
                                         rhs=W['wproj1'][:, 64:128],
                                         start=True, stop=True,
                                         skip_group_check=True)
                        tst = sb.tile([P, 68], bf16, name="tst1", tag="tst1")
                        nc.scalar.activation(out=tst[:, 0:64],
                                             in_=nb_[:, 3 * P:3 * P + 64],
                                             func=AF.Copy)
                        nc.vector.tensor_scalar_mul(tst[:, 64:68], posn[:], -1.0)
                        nc.gpsimd.dma_start(ts1sh[t * P:(t + 1) * P, 0:68], tst[:])
                        if DBG and t == 0:
                            nc.sync.dma_start(dbg8[:, :], tst[:])
                        nc.tensor.matmul(out=nb_[:, 3 * P + 64:4 * P],
                                         lhsT=xout_T[:, t * P:(t + 1) * P],
                                         rhs=W['wproj1'][:, 0:64],
                                         start=True, stop=True,
                                         skip_group_check=True)
                        xds = sb.tile([P, 68], bf16, name="xds1", tag="xds1")
                        nc.scalar.activation(out=xds[:, 0:64],
                                             in_=nb_[:, 3 * P + 64:4 * P],
                                             func=AF.Copy)
                        nc.vector.tensor_copy(out=xds[:, 64:68], in_=posn[:])
                        nc.gpsimd.dma_start(xdp[1][t * P:(t + 1) * P, 0:68], xds[:])
                    else:
                        nc.tensor.matmul(out=nb_[:, 3 * P:4 * P],
                                         lhsT=xout_T[:, t * P:(t + 1) * P],
                                         rhs=identb[:], start=True, stop=True,
                                         skip_group_check=True)
                        x2n = sb.tile([P, P], bf16, name="x2n", tag="x2n")
                        nc.scalar.activation(out=x2n[:], in_=nb_[:, 3 * P:4 * P],
                                             func=AF.Copy)
                        nc.tensor.matmul(out=ppool_t[:],
                                         lhsT=bpool_t[:, t * 64:(t + 1) * 64],
                                         rhs=x2n[:],
                                         start=(t == 0), stop=(t == T - 1))

            # layer 0
            tc.strict_bb_all_engine_barrier()
            layer(0, ts0, xdp[0], pos_own, xT_a, xT_b)
            # allgather ts1
            tc.strict_bb_all_engine_barrier()
            nc.gpsimd.collective_compute(
                "AllGather", ALU.bypass, replica_groups=[list(range(NC))],
                ins=[ts1sh.ap().opt()], outs=[ts1.ap().opt()])
            tc.strict_bb_all_engine_barrier()
            # layer 1 (+ pooling accumulation)
            ppool_t = ps_pool.tile([G_, P], f32, name="ppool", tag="ppool",
                                   space="PSUM")
            layer(1, ts1, xdp[1], posn_all, xT_b, xT_a)
            # pooling tail
            gss = sb.tile([G_, P], f32, name="gss", tag="gss")
            nc.vector.tensor_copy(out=gss[:], in_=ppool_t[:])
            nc.sync.dma_start(gs_in[:, :], gss[:])
            if DBG:
                nc.sync.dma_start(dbg9[:, :], gss[:])
            tc.strict_bb_all_engine_barrier()
            nc.gpsimd.collective_compute(
                "AllReduce", ALU.add, replica_groups=[list(range(NC))],
                ins=[gs_in.ap().opt()], outs=[gs_out.ap().opt()])
            tc.strict_bb_all_engine_barrier()
            gsr = sb.tile([G_, P], f32, name="gsr", tag="gsr")
            nc.sync.dma_start(gsr[:], gs_out[:, :])
            gm = sb.tile([G_, P], bf16, name="gm", tag="gm")
            nc.vector.tensor_scalar(out=gm[:], in0=gsr[:],
                                    scalar1=invcnt_t[:, 0:1], scalar2=0.0,
                                    op0=ALU.mult, op1=ALU.max)
            tb = ps_nd.tile([P, 512], f32, name="pnd", tag="pnd", space="PSUM")
            nc.tensor.matmul(out=tb[:, 0:G_], lhsT=gm[:],
                             rhs=identb[0:G_, 0:G_],
                             start=True, stop=True, skip_group_check=True)
            gT = sb.tile([P, G_], bf16, name="gT", tag="gT")
            nc.scalar.activation(out=gT[:], in_=tb[:, 0:G_], func=AF.Copy)
            nc.tensor.matmul(out=tb[:, G_:2 * G_], lhsT=W['wo1'][:], rhs=gT[:],
                             start=True, stop=True, skip_group_check=True)
            r1 = sb.tile([P, G_], bf16, name="r1", tag="r1")
            nc.scalar.activation(out=r1[:], in_=tb[:, G_:2 * G_], func=AF.Relu,
                                 bias=W['wo1b'][:, 0:1])
            nc.tensor.matmul(out=tb[0:32, 2 * G_:3 * G_], lhsT=W['wo2'][:],
                             rhs=r1[:], start=True, stop=True,
                             skip_group_check=True)
            o2 = sb.tile([32, G_], bf16, name="o2", tag="o2")
            with nc.allow_low_precision("final 32x64 to bf16 for PE transpose"):
                nc.vector.tensor_scalar_add(o2[:], tb[0:32, 2 * G_:3 * G_],
                                            W['wo2b'][:, 0:1])
            nc.tensor.matmul(out=tb[0:G_, 3 * G_:3 * G_ + 32], lhsT=o2[:],
                             rhs=identb[0:32, 0:32],
                             start=True, stop=True, skip_group_check=True)
            oT = sb.tile([G_, 32], f32, name="oT", tag="oT")
            nc.scalar.activation(out=oT[:], in_=tb[0:G_, 3 * G_:3 * G_ + 32],
                                 func=AF.Copy)
            nc.sync.dma_start(out_ext[:, :], oT[:])

    from concourse.library_overlay import lower_extended_insts
    lower_extended_insts(nc)
    return nc


def run(inputs, n_tiles_per_core, trace=False):
    st = host_prep(inputs, n_tiles_per_core)
    w = host_weights(inputs)
    SH, T = st['SH'], st['T']
    cb = (w['cb0'], w['cb1'])
    nc = build(st, cb)
    wt = {k: v for k, v in w.items() if k in WSPEC}
    in_maps = []
    for c in range(NC):
        m = dict(xT_full=st['xT'],
                 xT_own=np.ascontiguousarray(st['xT'][:, c * SH:(c + 1) * SH]),
                 pos_tf=st['pos_tf'],
                 pos_own=np.ascontiguousarray(
                     st['pos_tf'][:, c * T * 4:(c + 1) * T * 4]),
                 src_w=st['src_w'][c], dstl_w=st['dstl_w'][c],
                 nloc_t=st['nloc_t'][c], ea_t=st['ea_t'][c],
                 bpool=st['bpool'][c], invcnt=st['invcnt'],
                 iotab=st['iotab'])
        m.update(wt)
        in_maps.append(m)
    res = bass_utils.run_bass_kernel_spmd(nc, in_maps, core_ids=list(range(NC)),
                                          trace=trace)
    return res


def kernel(**inputs):
    n_tiles = math.ceil(inputs['x'].shape[0] / (P * NC))
    res = run(inputs, n_tiles)
    return res.results[0]['out']


# revision 22
# speedup vs baseline: 1.1660x; 1.1660x over previous
"""EGNN (2-layer, graph pooling) Trainium2 SPMD kernel over 8 NeuronCores.

v2: edges dst-sorted and sharded by destination-node range. Per 128-node
dst tile the kernel batch-gathers BOTH endpoint projections via indirect
DMA (src from a full projected table, dst from the core-local table),
builds the edge MLP in h'-major form on the TensorEngine (weights
stationary; per-block identity-matmuls transpose the gathered sums into
the PSUM accumulation), generates the scatter one-hot on-chip from local
dst indices, and scatter-adds with one matmul per 128-edge block. Matmul
operands are bf16 (fp32 PSUM accumulation). The layer-1 feature table is
exchanged with an AllGather; graph pooling uses one-hot matmuls and a
final AllReduce. Walrus in this environment accepts one sync-wait per
instruction, so a JSON-level pass splits multi-wait instructions onto
NoOp carriers.
"""
import sys
sys.path.insert(0, '/opt/trn_rl_repo')
import concourse.tile as tile_mod
from concourse.vector_clock import ScopedClock


def _patched_drain_and_barrier(self, tick_clock, wait_clock):
    nc = self.nc
    probe = nc.sync.nop(nofuse=True)
    wait_clock.add_sem_waits(probe.ins, ScopedClock({None: tick_clock.global_clock}))
    waits = list(probe.ins.sync_info.on_wait)
    probe.ins.sync_info.on_wait = []
    import concourse.mybir as mybir
    for w in waits:
        carrier = nc.sync.nop(nofuse=True)
        if carrier.ins.sync_info is None:
            carrier.ins.sync_info = mybir.SyncInfo(on_wait=[], on_update=[])
        carrier.ins.sync_info.on_wait = [w]
    nc.sync.drain()

    nc.all_engine_barrier()
    assert self.sems is not None
    popped = nc._tile_sem_poison_stack.pop()
    assert popped is self._sem_poison
    nc.clear_and_free_semaphores(list(self.sems.allocated().values()))
    nc.all_engine_barrier()


def apply_patch():
    tile_mod.TileContext._drain_and_barrier = _patched_drain_and_barrier


def _legalize_waits_json(mod: dict) -> dict:
    """Walrus in this env accepts at most ONE sync wait per instruction.
    Split extra waits onto same-engine NoOp carriers inserted just before."""
    n_new = [0]
    for fn in mod.get('functions', []):
        for blk in fn.get('blocks', []):
            insts = blk.get('instructions', [])
            out = []
            for inst in insts:
                si = inst.get('sync_info') or {}
                waits = si.get('on_wait') or []
                if len(waits) > 1:
                    eng = inst.get('engine')
                    for w in waits[:-1]:
                        n_new[0] += 1
                        out.append({
                            'debug': inst.get('debug', 0),
                            'engine': eng, 'ins': [], 'outs': [],
                            'name': 'I-waitfix-%d' % n_new[0],
                            'opcode': 'NoOp',
                            'sync_info': {'on_update': [], 'on_wait': [w]},
                        })
                    si['on_wait'] = [waits[-1]]
                out.append(inst)
            blk['instructions'] = out
    return mod


def apply_json_patch():
    import orjson
    import concourse.bass as bass_mod
    orig = bass_mod.Bass.to_json_bytes
    def to_json_bytes(self):
        raw = orig(self)
        mod = orjson.loads(raw)
        mod = _legalize_waits_json(mod)
        return orjson.dumps(mod)
    bass_mod.Bass.to_json_bytes = to_json_bytes


import math
import numpy as np
import ml_dtypes
import concourse.bass as bass
import concourse.mybir as mybir
from concourse.tile import TileContext
from concourse import bass_utils
from concourse.masks import make_identity
from concourse import library_config
apply_patch(); apply_json_patch()

f32 = mybir.dt.float32
bf16 = mybir.dt.bfloat16
i32 = mybir.dt.int32
AF = mybir.ActivationFunctionType
ALU = mybir.AluOpType
AX = mybir.AxisListType
P = 128
NC = 8
GRP = 4          # blocks per h PSUM group (GRP*128 f32 = one 2KB bank)
DBG = False

bft = ml_dtypes.bfloat16


def host_prep(inputs, n_tiles_per_core):
    SH = n_tiles_per_core * P
    NPAD = SH * NC
    T = n_tiles_per_core
    N = inputs['x'].shape[0]
    src = np.asarray(inputs['edge_index'][0], np.int64)
    dst = np.asarray(inputs['edge_index'][1], np.int64)
    ea = np.asarray(inputs['edge_attr'], np.float32)
    order = np.argsort(dst, kind='stable')
    src, dst, ea = src[order], dst[order], ea[order]
    core_of = dst // SH
    tile_of = (dst % SH) // P

    counts = np.zeros((NC, T), np.int64)
    for c in range(NC):
        m = core_of == c
        tl, cn = np.unique(tile_of[m], return_counts=True)
        counts[c, tl] = cn
    K = np.maximum(1, np.ceil(counts / P).astype(np.int64).max(axis=0))
    offs = np.concatenate([[0], np.cumsum(K)]).astype(np.int64)
    TOT = int(offs[-1])

    src_pm = np.zeros((NC, P, TOT), np.int32)          # global src node id
    dstl_pm = np.zeros((NC, P, TOT), np.int32)         # dst local to core
    nloc_t = np.full((NC, P, TOT), -1.0, np.float32)   # dst local to tile, pad -1
    src_w = np.zeros((NC, P, TOT * 8), np.int16)       # dma_gather 16-p wrap
    dstl_w = np.zeros((NC, P, TOT * 8), np.int16)
    ea_t = np.zeros((NC, 16, TOT * P), bft)
    for c in range(NC):
        m = core_of == c
        s_c, d_c, e_c, t_c = src[m], dst[m], ea[m], tile_of[m]
        for t in range(T):
            mt = t_c == t
            sc, dc, ec = s_c[mt], d_c[mt], e_c[mt]
            dloc_core = (dc - c * SH).astype(np.int64)
            nloc = dloc_core - t * P
            n_e = len(sc)
            for k in range(int(K[t])):
                blk = int(offs[t]) + k
                lo, hi = k * P, min((k + 1) * P, n_e)
                cnt = max(0, hi - lo)
                if cnt > 0:
                    src_pm[c, :cnt, blk] = sc[lo:hi]
                    dstl_pm[c, :cnt, blk] = dloc_core[lo:hi]
                    nloc_t[c, :cnt, blk] = nloc[lo:hi]
                    ea_t[c, :, blk * P: blk * P + cnt] = ec[lo:hi].T.astype(bft)
    for c in range(NC):
        for t in range(T):
            off, Kt = int(offs[t]), int(K[t])
            lin_s = src_pm[c][:, off:off + Kt].T.ravel()     # i = k*128+p
            lin_d = dstl_pm[c][:, off:off + Kt].T.ravel()
            bs = lin_s.reshape(Kt * 8, 16).T.astype(np.int16)
            bd = lin_d.reshape(Kt * 8, 16).T.astype(np.int16)
            for c8 in range(8):      # replicated per Q7 core stripe
                src_w[c][c8 * 16:(c8 + 1) * 16, off * 8:(off + Kt) * 8] = bs
                dstl_w[c][c8 * 16:(c8 + 1) * 16, off * 8:(off + Kt) * 8] = bd
    s4_h = np.zeros((NC, P, TOT * P), bft)
    for c in range(NC):
        a = np.zeros((P, TOT, P), np.float32)
        v = nloc_t[c] >= 0
        ii = np.clip(nloc_t[c].astype(np.int64), 0, 127)
        np.put_along_axis(a, ii[..., None],
                          np.where(v, 1.0, 0.0)[..., None], axis=2)
        s4_h[c] = a.reshape(P, TOT * P).astype(bft)
    # pooling one-hots + counts
    batch = np.asarray(inputs['batch'], np.int64)
    G_ = 64
    bpool = np.zeros((NC, P, T * 64), bft)
    for c in range(NC):
        base = c * SH
        for t in range(T):
            n0 = base + t * P
            n1 = min(n0 + P, N)
            if n1 > n0:
                rows = np.arange(n1 - n0)
                bpool[c, rows, t * 64 + batch[n0:n1]] = 1.0
    cnts = np.bincount(batch, minlength=G_).astype(np.float32)
    invcnt = (1.0 / np.maximum(cnts, 1.0)).reshape(G_, 1)

    # node data
    xf = np.zeros((NPAD, P), np.float32)
    xf[:N] = np.asarray(inputs['x'], np.float32)
    xT = np.ascontiguousarray(xf.T.astype(bft))            # (128, NPAD)
    pf = np.zeros((NPAD, 4), np.float32)
    pf[:N, 0:3] = np.asarray(inputs['pos'], np.float32)
    # tiled pos: (P, nt*4) with [p, 4t:4t+4] = pos[t*128+p]
    NT_FULL = NPAD // P
    pos_tf = np.ascontiguousarray(
        pf.reshape(NT_FULL, P, 4).transpose(1, 0, 2).reshape(P, NT_FULL * 4))
    iotab = np.tile(np.arange(P, dtype=np.float32), (P, 1))
    return dict(SH=SH, NPAD=NPAD, T=T, K=[int(k) for k in K],
                offs=[int(o) for o in offs], TOT=TOT, src_pm=src_pm,
                dstl_pm=dstl_pm, src_w=src_w, dstl_w=dstl_w,
                nloc_t=nloc_t, s4_h=s4_h, ea_t=ea_t, bpool=bpool,
                invcnt=invcnt, xT=xT, pos_tf=pos_tf, iotab=iotab)


def host_weights(inputs):
    w = {}
    for L in range(2):
        mw = np.asarray(inputs[f'l{L}_mlp_w'], np.float32)
        w[f'wproj{L}'] = np.concatenate([mw[0:128], mw[128:256]],
                                        axis=1).astype(bft)     # [dst|src]
        w[f'wea{L}'] = np.ascontiguousarray(mw[256:272]).astype(bft)
        w[f'wrrep{L}'] = np.tile(mw[272:273], (P, 1)).astype(bft)
        ew = np.asarray(inputs[f'l{L}_edge_w'], np.float32)
        eb = np.asarray(inputs[f'l{L}_edge_b'], np.float32)
        we1s = np.zeros((65, 64), np.float32)
        we1s[0:64] = ew; we1s[64] = eb
        w[f'we1s{L}'] = we1s.astype(bft)
        cw = np.asarray(inputs[f'l{L}_coord_w'], np.float32)
        w[f'cwrep{L}'] = np.tile(cw[:, 0][None, :], (P, 1)).astype(bft)
        w[f'cb{L}'] = float(np.asarray(inputs[f'l{L}_coord_b'], np.float32)[0])
        n1 = np.asarray(inputs[f'l{L}_node_w1'], np.float32)
        w[f'wn1x{L}'] = np.ascontiguousarray(n1[0:128]).astype(bft)
        w[f'wn1a{L}'] = np.ascontiguousarray(n1[128:192]).astype(bft)
        w[f'nb1_{L}'] = np.asarray(
            inputs[f'l{L}_node_b1'], np.float32).reshape(64, 1)  # bias col
        w[f'wn2b{L}'] = np.concatenate(
            [np.asarray(inputs[f'l{L}_node_w2'], np.float32),
             np.asarray(inputs[f'l{L}_node_b2'], np.float32)[None, :]],
            0).astype(bft)
    w['wo1'] = np.asarray(inputs['out_w1'], np.float32).astype(bft)
    w['wo1b'] = np.asarray(inputs['out_b1'], np.float32).reshape(P, 1)
    w['wo2'] = np.asarray(inputs['out_w2'], np.float32).astype(bft)
    w['wo2b'] = np.asarray(inputs['out_b2'], np.float32).reshape(32, 1)
    return w


# name -> (shape, dtype); cb0/cb1 are python floats, not tensors
WSPEC = dict(wproj0=((P, P), bf16), wproj1=((P, P), bf16),
             wea0=((16, 64), bf16), wea1=((16, 64), bf16),
             wrrep0=((P, 64), bf16), wrrep1=((P, 64), bf16),
             we1s0=((65, 64), bf16), we1s1=((65, 64), bf16),
             cwrep0=((P, 64), bf16), cwrep1=((P, 64), bf16),
             wn1x0=((P, 64), bf16), wn1x1=((P, 64), bf16),
             wn1a0=((64, 64), bf16), wn1a1=((64, 64), bf16),
             nb1_0=((64, 1), f32), nb1_1=((64, 1), f32),
             wn2b0=((65, P), bf16), wn2b1=((65, P), bf16),
             wo1=((P, P), bf16), wo1b=((P, 1), f32),
             wo2=((P, 32), bf16), wo2b=((32, 1), f32))


def _bc_k(ap2d, Kt):
    """(P, C) -> (P, Kt, C) broadcast view (k stride 0)."""
    a = ap2d.rearrange("p (k c) -> p k c", k=1)
    new = [list(d) for d in a.ap]
    new[1] = [0, Kt]
    return bass.AP(a.tensor, a.offset, new)


def _bc_c(ap2d, C):
    """(P, Kt) -> (P, Kt, C) broadcast view (c stride 0)."""
    a = ap2d.rearrange("p (k o) -> p k o", o=1)
    new = [list(d) for d in a.ap]
    new[2] = [0, C]
    return bass.AP(a.tensor, a.offset, new)


def build(st, cb):
    SH, NPAD, T, K, offs, TOT = (st['SH'], st['NPAD'], st['T'], st['K'],
                                 st['offs'], st['TOT'])
    NT_FULL = NPAD // P
    G_, OUT = 64, 32
    KMAX = max(K)

    nc = bass.Bass("TRN2")
    dram = {}
    def din(name, shape, dt=f32):
        dram[name] = nc.dram_tensor(name, shape, dt, kind="ExternalInput")
        return dram[name]

    xT_full = din('xT_full', (P, NPAD), bf16)
    xT_own = din('xT_own', (P, SH), bf16)
    pos_tf_d = din('pos_tf', (P, NT_FULL * 4))
    pos_own_d = din('pos_own', (P, T * 4))
    i16 = mybir.dt.int16
    srcw_d = din('src_w', (P, TOT * 8), i16)
    dstlw_d = din('dstl_w', (P, TOT * 8), i16)
    nloc_d = din('nloc_t', (P, TOT))
    s4_d = din('s4_h', (P, TOT * P), bf16)
    ea_td = din('ea_t', (16, TOT * P), bf16)
    bpool_d = din('bpool', (P, T * 64), bf16)
    invcnt_d = din('invcnt', (G_, 1))
    iotab_d = din('iotab', (P, P))
    for n, (shp, dt) in WSPEC.items():
        din(n, shp, dt)
    out_ext = nc.dram_tensor('out', (G_, OUT), f32, kind="ExternalOutput")

    ts0 = nc.dram_tensor('ts0', (NPAD, P), bf16, kind='ExternalOutput')
    ts1sh = nc.dram_tensor('ts1sh', (SH, P), bf16)
    ts1 = nc.dram_tensor('ts1', (NPAD, P), bf16, addr_space="Shared")
    xdp = [nc.dram_tensor('xdp0', (SH, P), bf16, kind='ExternalOutput'),
           nc.dram_tensor('xdp1', (SH, P), bf16)]
    if DBG:
        dbg1 = nc.dram_tensor('dbg1', (P, 68 * 4), bf16, kind='ExternalOutput')
        dbg2 = nc.dram_tensor('dbg2', (P, 64 + P + 64), bf16,
                              kind='ExternalOutput')
        dbg3 = nc.dram_tensor('dbg3', (65, 17 * P), bf16, kind='ExternalOutput')
        dbg5 = nc.dram_tensor('dbg5', (P, 64), bf16, kind='ExternalOutput')
        dbg6 = nc.dram_tensor('dbg6', (P, 4), f32, kind='ExternalOutput')
        dbg7 = nc.dram_tensor('dbg7', (P, P), bf16, kind='ExternalOutput')
        dbg8 = nc.dram_tensor('dbg8', (P, 68), bf16, kind='ExternalOutput')
        dbg9 = nc.dram_tensor('dbg9', (64, P), f32, kind='ExternalOutput')
        dbg10 = nc.dram_tensor('dbg10', (P, 17 * 68), bf16,
                               kind='ExternalOutput')
        dbg11 = nc.dram_tensor('dbg11', (P, 17 * P), bf16,
                               kind='ExternalOutput')
        dbg12 = nc.dram_tensor('dbg12', (P, 17 * 64), bf16,
                               kind='ExternalOutput')
        dbg13 = nc.dram_tensor('dbg13', (P, 17 * 68), bf16,
                               kind='ExternalOutput')
        dbg14 = nc.dram_tensor('dbg14', (P, 17), f32, kind='ExternalOutput')
        dbg15 = nc.dram_tensor('dbg15', (P, 17 * 68), bf16,
                               kind='ExternalOutput')
    gs_in = nc.dram_tensor('gs_in', (G_, P), f32)
    gs_out = nc.dram_tensor('gs_out', (G_, P), f32, addr_space="Shared")

    with TileContext(nc) as tc:
        with (tc.tile_pool(name="pers", bufs=1) as pers,
              tc.tile_pool(name="sa", bufs=3) as sa,
              tc.tile_pool(name="sb", bufs=2) as sb,
              tc.tile_pool(name="sg", bufs=2) as sgp,
              tc.tile_pool(name="ph", bufs=2, space="PSUM") as ps_h,
              tc.tile_pool(name="pe1", bufs=2, space="PSUM") as ps_e1,
              tc.tile_pool(name="pagg", bufs=1, space="PSUM") as ps_agg,
              tc.tile_pool(name="pnd", bufs=2, space="PSUM") as ps_nd,
              tc.tile_pool(name="ppool", bufs=1, space="PSUM") as ps_pool):

            nc.gpsimd.load_library(library_config.mlp)
            identb = pers.tile([P, P], bf16, name="identb", tag="identb")
            make_identity(nc, identb[:])
            invcnt_t = pers.tile([G_, 1], f32, name="invc", tag="invc")
            nc.sync.dma_start(invcnt_t[:], invcnt_d[:, :])
            W = {}
            for n, (shp, dt) in WSPEC.items():
                W[n] = pers.tile(list(shp), dt, name="w_" + n, tag="w_" + n)
                nc.sync.dma_start(W[n][:], dram[n][:, :])
            xT_a = pers.tile([P, SH], bf16, name="xT_a", tag="xT_a")
            nc.sync.dma_start(xT_a[:], xT_own[:, :])
            xT_b = pers.tile([P, SH], bf16, name="xT_b", tag="xT_b")
            pos_own = pers.tile([P, T * 4], f32, name="pos_own", tag="pos_own")
            nc.sync.dma_start(pos_own[:], pos_own_d[:, :])
            posn_all = pers.tile([P, T * 4], f32, name="posn_all", tag="posn_all")
            pos_tfull = pers.tile([P, NT_FULL * 4], f32, name="pos_tf",
                                  tag="pos_tf")
            nc.sync.dma_start(pos_tfull[:], pos_tf_d[:, :])
            bpool_t = pers.tile([P, T * 64], bf16, name="bpool", tag="bpool")
            nc.sync.dma_start(bpool_t[:], bpool_d[:, :])
            srcw_t = pers.tile([P, TOT * 8], i16, name="srcw", tag="srcw")
            nc.sync.dma_start(srcw_t[:], srcw_d[:, :])
            dstlw_t = pers.tile([P, TOT * 8], i16, name="dstlw", tag="dstlw")
            nc.sync.dma_start(dstlw_t[:], dstlw_d[:, :])
            # persistent double-buffered tiles with constant regions
            hTs_b = [pers.tile([65, KMAX * P], bf16, name=f"hTs{i}", tag=f"hTs{i}")
                     for i in range(2)]
            scat_b = [pers.tile([P, KMAX * 68], bf16, name=f"scat{i}",
                                tag=f"scat{i}") for i in range(2)]
            zst_b = [pers.tile([65, P], bf16, name=f"zst{i}", tag=f"zst{i}")
                     for i in range(2)]
            for i in range(2):
                nc.vector.memset(hTs_b[i][64:65, :], 1.0)
                sc3i = scat_b[i][:].rearrange("p (k c) -> p k c", c=68)
                nc.vector.memset(sc3i[:, :, 64:65], 1.0)
                nc.vector.memset(zst_b[i][64:65, :], 1.0)

            # ---------------- stage A: full-N ts0 table ----------------
            for ti in range(NT_FULL):
                xt = sa.tile([P, P], bf16, name="ax", tag="ax")
                nc.sync.dma_start(xt[:], xT_full[:, ti * P:(ti + 1) * P])
                nb_ = ps_nd.tile([P, 512], f32, name="pnd", tag="pnd",
                                 space="PSUM")
                nc.tensor.matmul(out=nb_[:, 0:64], lhsT=xt[:],
                                 rhs=W['wproj0'][:, 64:128],
                                 start=True, stop=True, skip_group_check=True)
                tst = sa.tile([P, 68], bf16, name="tst", tag="tst")
                nc.scalar.activation(out=tst[:, 0:64], in_=nb_[:, 0:64],
                                     func=AF.Copy)
                nc.vector.tensor_scalar_mul(
                    tst[:, 64:68], pos_tfull[:, ti * 4:(ti + 1) * 4], -1.0)
                nc.sync.dma_start(ts0[ti * P:(ti + 1) * P, 0:68], tst[:])

            # ---------------- stage A-own: xdp0 ------------------------
            for t in range(T):
                nb_ = ps_nd.tile([P, 512], f32, name="pnd", tag="pnd",
                                 space="PSUM")
                nc.tensor.matmul(out=nb_[:, 0:64], lhsT=xT_a[:, t * P:(t + 1) * P],
                                 rhs=W['wproj0'][:, 0:64], start=True, stop=True,
                                 skip_group_check=True)
                xds = sa.tile([P, 68], bf16, name="xds", tag="xds")
                nc.scalar.activation(out=xds[:, 0:64], in_=nb_[:, 0:64],
                                     func=AF.Copy)
                nc.vector.tensor_copy(out=xds[:, 64:68],
                                      in_=pos_own[:, t * 4:(t + 1) * 4])
                nc.sync.dma_start(xdp[0][t * P:(t + 1) * P, 0:68], xds[:])

            nidx_regs = {}
            def nidx_reg(n):
                if n not in nidx_regs:
                    nidx_regs[n] = nc.gpsimd.to_reg(n)
                return nidx_regs[n]

            # ---------------- edge + node stage, per layer --------------
            def layer(L, ts_dram, xdp_dram, pos_tile_of, xin_T, xout_T):
                sfx = str(L)
                for t in range(T):
                    Kt = K[t]
                    off = offs[t]
                    ng = (Kt + GRP - 1) // GRP
                    buf = (L * T + t) % 2
                    hT_s = hTs_b[buf]
                    scat = scat_b[buf]
                    zst = zst_b[buf]
                    sc3 = scat[:].rearrange("p (k c) -> p k c", c=68)

                    eat = sgp.tile([16, KMAX * P], bf16, name="eat", tag="eat")
                    nc.sync.dma_start(eat[:, 0:Kt * P],
                                        ea_td[:, off * P:(off + Kt) * P])
                    g_src = sgp.tile([P, KMAX * P], bf16, name="gsrc", tag="gsrc")
                    gs3 = g_src[:].rearrange("p (k c) -> p k c", c=P)
                    g_dst = sgp.tile([P, KMAX * P], bf16, name="gdst", tag="gdst")
                    gd3 = g_dst[:].rearrange("p (k c) -> p k c", c=P)
                    for g3v, tbl, idxt in ((gs3, ts_dram, srcw_t),
                                           (gd3, xdp_dram, dstlw_t)):
                        k0 = 0
                        while k0 < Kt:
                            kn = min(8, Kt - k0)      # <=1024 descs per ring
                            nc.gpsimd.dma_gather(
                                out_ap=g3v[:, k0:k0 + kn, :], in_ap=tbl[:, :],
                                idxs_ap=idxt[:, (off + k0) * 8:(off + k0 + kn) * 8],
                                num_idxs=kn * P, num_idxs_reg=nidx_reg(kn * P),
                                elem_size=P)
                            k0 += kn

                    gsum = sb.tile([P, KMAX * 68], bf16, name="gsum", tag="gsum")
                    g3s = gsum[:].rearrange("p (k c) -> p k c", c=68)
                    nc.vector.tensor_tensor(out=g3s[:, 0:Kt, :],
                                            in0=gs3[:, 0:Kt, 0:68],
                                            in1=gd3[:, 0:Kt, 0:68], op=ALU.add)
                    g3 = gsum[:].rearrange("p (k c) -> p k c", c=68)
                    # radial = sum(diff^2), diff = gsum[:, k, 64:67]
                    dsq = sb.tile([P, KMAX * 3], f32, name="dsq", tag="dsq")
                    q3 = dsq[:].rearrange("p (k c) -> p k c", c=3)
                    nc.vector.tensor_tensor(out=q3[:, 0:Kt, :],
                                            in0=g3[:, 0:Kt, 64:67],
                                            in1=g3[:, 0:Kt, 64:67], op=ALU.mult)
                    radst = sb.tile([P, KMAX], f32, name="rad", tag="rad")
                    nc.vector.tensor_reduce(
                        out=radst[:, 0:Kt].rearrange("p (k o) -> p k o", o=1),
                        in_=q3[:, 0:Kt, :], axis=AX.X, op=ALU.add)
                    # gsum2 = gsum[:, k, 0:64] + radial * w_r  (fused per block)
                    gsum2 = sb.tile([P, KMAX * 64], bf16, name="gsum2",
                                    tag="gsum2")
                    for k in range(Kt):
                        nc.vector.scalar_tensor_tensor(
                            out=gsum2[:, k * 64:(k + 1) * 64],
                            in0=W['wrrep' + sfx][:],
                            scalar=radst[:, k:k + 1],
                            in1=gsum[:, k * 68:k * 68 + 64],
                            op0=ALU.mult, op1=ALU.add)
                    # scatter one-hot from host
                    s4 = sb.tile([P, KMAX * P], bf16, name="s4", tag="s4")
                    nc.sync.dma_start(s4[:, 0:Kt * P],
                                      s4_d[:, off * P:(off + Kt) * P])
                    # h'-major h: psum groups of GRP blocks
                    for g in range(ng):
                        nb = min(GRP, Kt - g * GRP)
                        hp = ps_h.tile([64, GRP * P], f32, name="hp", tag="hp",
                                       space="PSUM")
                        for j in range(nb):
                            k = g * GRP + j
                            nc.tensor.matmul(
                                out=hp[:, j * P:(j + 1) * P],
                                lhsT=gsum2[:, k * 64:(k + 1) * 64],
                                rhs=identb[:],
                                start=True, stop=False)
                            nc.tensor.matmul(
                                out=hp[:, j * P:(j + 1) * P],
                                lhsT=W['wea' + sfx][:],
                                rhs=eat[:, k * P:(k + 1) * P],
                                start=False, stop=True)
                        nc.scalar.activation(
                            out=hT_s[0:64, g * GRP * P:(g * GRP + nb) * P],
                            in_=hp[:, 0:nb * P], func=AF.Silu)
                    # e1 = silu(hT @ we1s) edge-major, into scat[:, :, 0:64]
                    for g in range(ng):
                        nb = min(GRP, Kt - g * GRP)
                        ep = ps_e1.tile([P, GRP * 64], f32, name="ep", tag="ep",
                                        space="PSUM")
                        for j in range(nb):
                            k = g * GRP + j
                            nc.tensor.matmul(out=ep[:, j * 64:(j + 1) * 64],
                                             lhsT=hT_s[:, k * P:(k + 1) * P],
                                             rhs=W['we1s' + sfx][:],
                                             start=True, stop=True)
                        e3 = ep[:, 0:nb * 64].rearrange("p (k c) -> p k c", c=64)
                        nc.scalar.activation(
                            out=sc3[:, g * GRP:g * GRP + nb, 0:64],
                            in_=e3[:, :, :], func=AF.Silu)
                    if L == 0:
                        # sgate = silu(e1 @ cw + cb); coord cols
                        e1m = sb.tile([P, KMAX * 64], bf16, name="e1m",
                                      tag="e1m")
                        m3 = e1m[:].rearrange("p (k c) -> p k c", c=64)
                        nc.vector.tensor_tensor(
                            out=m3[:, 0:Kt, :], in0=sc3[:, 0:Kt, 0:64],
                            in1=_bc_k(W['cwrep' + sfx][:], Kt), op=ALU.mult)
                        sgt = sb.tile([P, KMAX], f32, name="sgt", tag="sgt")
                        nc.vector.tensor_reduce(
                            out=sgt[:, 0:Kt].rearrange("p (k o) -> p k o", o=1),
                            in_=m3[:, 0:Kt, :], axis=AX.X, op=ALU.add)
                        sgs = sb.tile([P, KMAX], bf16, name="sgs", tag="sgs")
                        nc.scalar.activation(out=sgs[:, 0:Kt], in_=sgt[:, 0:Kt],
                                             func=AF.Silu, bias=cb[L])
                        nc.vector.tensor_tensor(out=sc3[:, 0:Kt, 65:68],
                                                in0=g3[:, 0:Kt, 64:67],
                                                in1=_bc_c(sgs[:, 0:Kt], 3),
                                                op=ALU.mult)
                    # scatter-add into node aggregate
                    NAGG = 68 if L == 0 else 65
                    pagg = ps_agg.tile([P, 68], f32, name="pagg", tag="pagg",
                                       space="PSUM")
                    for k in range(Kt):
                        nc.tensor.matmul(out=pagg[:, 0:NAGG],
                                         lhsT=s4[:, k * P:(k + 1) * P],
                                         rhs=scat[:, k * 68:k * 68 + NAGG],
                                         start=(k == 0), stop=(k == Kt - 1))
                    if DBG and L == 0 and t == 0:
                        nc.sync.dma_start(dbg10[:, 0:Kt * 68],
                                          scat[:, 0:Kt * 68])
                        nc.sync.dma_start(dbg11[:, 0:Kt * P], s4[:, 0:Kt * P])
                        nc.sync.dma_start(dbg12[:, 0:Kt * 64],
                                          gsum2[:, 0:Kt * 64])
                        nc.sync.dma_start(dbg13[:, 0:Kt * 68],
                                          gsum[:, 0:Kt * 68])
                        nc.sync.dma_start(dbg14[:, 0:Kt], radst[:, 0:Kt])
                        nc.sync.dma_start(dbg1[:, 0:68], g_src[:, 0:68])
                        nc.sync.dma_start(dbg1[:, 68:136], g_dst[:, 0:68])
                        nc.sync.dma_start(dbg15[:, 0:Kt * 68],
                                          gs3[:, 0:Kt, 0:68])
                        nc.sync.dma_start(dbg1[:, 136:204], gsum[:, 0:68])
                        nc.sync.dma_start(dbg1[:, 204:272], scat[:, 0:68])
                        nc.sync.dma_start(dbg2[:, 0:64], gsum2[:, 0:64])
                        nc.sync.dma_start(dbg2[:, 64:64 + P], s4[:, 0:P])
                        nc.sync.dma_start(dbg2[:, 64 + P:64 + P + 64],
                                          e1m[:, 0:64])
                        nc.sync.dma_start(dbg3[:, :], hT_s[:, 0:17 * P])
                    # ---- node stage ----
                    eagg = sb.tile([P, 64], bf16, name="eagg", tag="eagg")
                    nc.scalar.activation(out=eagg[:], in_=pagg[:, 0:64],
                                         func=AF.Copy)
                    if L == 0:
                        deg1 = sb.tile([P, 1], f32, name="deg", tag="deg")
                        nc.vector.tensor_scalar_max(deg1[:], pagg[:, 64:65], 1.0)
                        inv = sb.tile([P, 1], f32, name="inv", tag="inv")
                        nc.vector.reciprocal(out=inv[:], in_=deg1[:])
                        posn = sb.tile([P, 4], f32, name="posn", tag="posn")
                        nc.vector.tensor_scalar_mul(posn[:, 0:3], pagg[:, 65:68],
                                                    inv[:, 0:1])
                        nc.vector.tensor_tensor(
                            out=posn[:, 0:3], in0=posn[:, 0:3],
                            in1=pos_tile_of[:, t * 4:t * 4 + 3], op=ALU.add)
                        nc.vector.memset(posn[:, 3:4], 0.0)
                    nb_ = ps_nd.tile([P, 512], f32, name="pnd", tag="pnd",
                                     space="PSUM")
                    nc.tensor.matmul(out=nb_[0:64, 0:P], lhsT=eagg[:],
                                     rhs=identb[:], start=True, stop=True,
                                     skip_group_check=True)
                    eaT = sb.tile([64, P], bf16, name="eaT", tag="eaT")
                    nc.scalar.activation(out=eaT[:], in_=nb_[0:64, 0:P],
                                         func=AF.Copy)
                    nc.tensor.matmul(out=nb_[0:64, P:2 * P],
                                     lhsT=W['wn1x' + sfx][:],
                                     rhs=xin_T[:, t * P:(t + 1) * P],
                                     start=True, stop=False,
                                     skip_group_check=True)
                    nc.tensor.matmul(out=nb_[0:64, P:2 * P],
                                     lhsT=W['wn1a' + sfx][:],
                                     rhs=eaT[:], start=False, stop=True,
                                     skip_group_check=True)
                    nc.scalar.activation(out=zst[0:64, :], in_=nb_[0:64, P:2 * P],
                                         func=AF.Silu,
                                         bias=W['nb1_' + sfx][:, 0:1])
                    nc.tensor.matmul(out=nb_[:, 2 * P:3 * P],
                                     lhsT=W['wn2b' + sfx][:],
                                     rhs=zst[:], start=True, stop=True,
                                     skip_group_check=True)
                    nc.scalar.activation(out=xout_T[:, t * P:(t + 1) * P],
                                         in_=nb_[:, 2 * P:3 * P], func=AF.Copy)
                    if DBG and L == 0 and t == 0:
                        nc.sync.dma_start(dbg5[:, :], eagg[:])
                        nc.sync.dma_start(dbg6[:, :], posn[:])
                        nc.sync.dma_start(dbg7[:, :],
                                          xout_T[:, t * P:(t + 1) * P])
                    if L == 0:
                        nc.tensor.matmul(out=nb_[:, 3 * P:3 * P + 64],
                                         lhsT=xout_T[:, t * P:(t + 1) * P],
                                         rhs=W['wproj1'][:, 64:128],
                                         start=True, stop=True,
                                         skip_group_check=True)
                        tst = sb.tile([P, 68], bf16, name="tst1", tag="tst1")
                        nc.scalar.activation(out=tst[:, 0:64],
                                             in_=nb_[:, 3 * P:3 * P + 64],
                                             func=AF.Copy)
                        nc.vector.tensor_scalar_mul(tst[:, 64:68], posn[:], -1.0)
                        nc.sync.dma_start(ts1sh[t * P:(t + 1) * P, 0:68], tst[:])
                        if DBG and t == 0:
                            nc.sync.dma_start(dbg8[:, :], tst[:])
                        nc.tensor.matmul(out=nb_[:, 3 * P + 64:4 * P],
                                         lhsT=xout_T[:, t * P:(t + 1) * P],
                                         rhs=W['wproj1'][:, 0:64],
                                         start=True, stop=True,
                                         skip_group_check=True)
                        xds = sb.tile([P, 68], bf16, name="xds1", tag="xds1")
                        nc.scalar.activation(out=xds[:, 0:64],
                                             in_=nb_[:, 3 * P + 64:4 * P],
                                             func=AF.Copy)
                        nc.vector.tensor_copy(out=xds[:, 64:68], in_=posn[:])
                        nc.sync.dma_start(xdp[1][t * P:(t + 1) * P, 0:68], xds[:])
                    else:
                        nc.tensor.matmul(out=nb_[:, 3 * P:4 * P],
                                         lhsT=xout_T[:, t * P:(t + 1) * P],
                                         rhs=identb[:], start=True, stop=True,
                                         skip_group_check=True)
                        x2n = sb.tile([P, P], bf16, name="x2n", tag="x2n")
                        nc.scalar.activation(out=x2n[:], in_=nb_[:, 3 * P:4 * P],
                                             func=AF.Copy)
                        nc.tensor.matmul(out=ppool_t[:],
                                         lhsT=bpool_t[:, t * 64:(t + 1) * 64],
                                         rhs=x2n[:],
                                         start=(t == 0), stop=(t == T - 1))

            # layer 0
            tc.strict_bb_all_engine_barrier()
            layer(0, ts0, xdp[0], pos_own, xT_a, xT_b)
            # allgather ts1
            tc.strict_bb_all_engine_barrier()
            nc.gpsimd.collective_compute(
                "AllGather", ALU.bypass, replica_groups=[list(range(NC))],
                ins=[ts1sh.ap().opt()], outs=[ts1.ap().opt()])
            tc.strict_bb_all_engine_barrier()
            # layer 1 (+ pooling accumulation)
            ppool_t = ps_pool.tile([G_, P], f32, name="ppool", tag="ppool",
                                   space="PSUM")
            layer(1, ts1, xdp[1], posn_all, xT_b, xT_a)
            # pooling tail
            gss = sb.tile([G_, P], f32, name="gss", tag="gss")
            nc.vector.tensor_copy(out=gss[:], in_=ppool_t[:])
            nc.sync.dma_start(gs_in[:, :], gss[:])
            if DBG:
                nc.sync.dma_start(dbg9[:, :], gss[:])
            tc.strict_bb_all_engine_barrier()
            nc.gpsimd.collective_compute(
                "AllReduce", ALU.add, replica_groups=[list(range(NC))],
                ins=[gs_in.ap().opt()], outs=[gs_out.ap().opt()])
            tc.strict_bb_all_engine_barrier()
            gsr = sb.tile([G_, P], f32, name="gsr", tag="gsr")
            nc.sync.dma_start(gsr[:], gs_out[:, :])
            gm = sb.tile([G_, P], bf16, name="gm", tag="gm")
            nc.vector.tensor_scalar(out=gm[:], in0=gsr[:],
                                    scalar1=invcnt_t[:, 0:1], scalar2=0.0,
                                    op0=ALU.mult, op1=ALU.max)
            tb = ps_nd.tile([P, 512], f32, name="pnd", tag="pnd", space="PSUM")
            nc.tensor.matmul(out=tb[:, 0:G_], lhsT=gm[:],
                             rhs=identb[0:G_, 0:G_],
                             start=True, stop=True, skip_group_check=True)
            gT = sb.tile([P, G_], bf16, name="gT", tag="gT")
            nc.scalar.activation(out=gT[:], in_=tb[:, 0:G_], func=AF.Copy)
            nc.tensor.matmul(out=tb[:, G_:2 * G_], lhsT=W['wo1'][:], rhs=gT[:],
                             start=True, stop=True, skip_group_check=True)
            r1 = sb.tile([P, G_], bf16, name="r1", tag="r1")
            nc.scalar.activation(out=r1[:], in_=tb[:, G_:2 * G_], func=AF.Relu,
                                 bias=W['wo1b'][:, 0:1])
            nc.tensor.matmul(out=tb[0:32, 2 * G_:3 * G_], lhsT=W['wo2'][:],
                             rhs=r1[:], start=True, stop=True,
                             skip_group_check=True)
            o2 = sb.tile([32, G_], bf16, name="o2", tag="o2")
            with nc.allow_low_precision("final 32x64 to bf16 for PE transpose"):
                nc.vector.tensor_scalar_add(o2[:], tb[0:32, 2 * G_:3 * G_],
                                            W['wo2b'][:, 0:1])
            nc.tensor.matmul(out=tb[0:G_, 3 * G_:3 * G_ + 32], lhsT=o2[:],
                             rhs=identb[0:32, 0:32],
                             start=True, stop=True, skip_group_check=True)
            oT = sb.tile([G_, 32], f32, name="oT", tag="oT")
            nc.scalar.activation(out=oT[:], in_=tb[0:G_, 3 * G_:3 * G_ + 32],
                                 func=AF.Copy)
            nc.sync.dma_start(out_ext[:, :], oT[:])

    from concourse.library_overlay import lower_extended_insts
    lower_extended_insts(nc)
    return nc


def run(inputs, n_tiles_per_core, trace=False):
    st = host_prep(inputs, n_tiles_per_core)
    w = host_weights(inputs)
    SH, T = st['SH'], st['T']
    cb = (w['cb0'], w['cb1'])
    nc = build(st, cb)
    wt = {k: v for k, v in w.items() if k in WSPEC}
    in_maps = []
    for c in range(NC):
        m = dict(xT_full=st['xT'],
                 xT_own=np.ascontiguousarray(st['xT'][:, c * SH:(c + 1) * SH]),
                 pos_tf=st['pos_tf'],
                 pos_own=np.ascontiguousarray(
                     st['pos_tf'][:, c * T * 4:(c + 1) * T * 4]),
                 src_w=st['src_w'][c], dstl_w=st['dstl_w'][c],
                 nloc_t=st['nloc_t'][c], s4_h=st['s4_h'][c],
                 ea_t=st['ea_t'][c],
                 bpool=st['bpool'][c], invcnt=st['invcnt'],
                 iotab=st['iotab'])
        m.update(wt)
        in_maps.append(m)
    res = bass_utils.run_bass_kernel_spmd(nc, in_maps, core_ids=list(range(NC)),
                                          trace=trace)
    return res


def kernel(**inputs):
    n_tiles = math.ceil(inputs['x'].shape[0] / (P * NC))
    res = run(inputs, n_tiles)
    return res.results[0]['out']


# revision 24
# speedup vs baseline: 2.0767x; 1.7811x over previous
"""EGNN (2-layer, graph pooling) Trainium2 SPMD kernel over 8 NeuronCores.

v2: edges dst-sorted and sharded by destination-node range. Per 128-node
dst tile the kernel batch-gathers BOTH endpoint projections via indirect
DMA (src from a full projected table, dst from the core-local table),
builds the edge MLP in h'-major form on the TensorEngine (weights
stationary; per-block identity-matmuls transpose the gathered sums into
the PSUM accumulation), generates the scatter one-hot on-chip from local
dst indices, and scatter-adds with one matmul per 128-edge block. Matmul
operands are bf16 (fp32 PSUM accumulation). The layer-1 feature table is
exchanged with an AllGather; graph pooling uses one-hot matmuls and a
final AllReduce. Walrus in this environment accepts one sync-wait per
instruction, so a JSON-level pass splits multi-wait instructions onto
NoOp carriers.
"""
import sys
sys.path.insert(0, '/opt/trn_rl_repo')
import concourse.tile as tile_mod
from concourse.vector_clock import ScopedClock


def _patched_drain_and_barrier(self, tick_clock, wait_clock):
    nc = self.nc
    probe = nc.sync.nop(nofuse=True)
    wait_clock.add_sem_waits(probe.ins, ScopedClock({None: tick_clock.global_clock}))
    waits = list(probe.ins.sync_info.on_wait)
    probe.ins.sync_info.on_wait = []
    import concourse.mybir as mybir
    for w in waits:
        carrier = nc.sync.nop(nofuse=True)
        if carrier.ins.sync_info is None:
            carrier.ins.sync_info = mybir.SyncInfo(on_wait=[], on_update=[])
        carrier.ins.sync_info.on_wait = [w]
    nc.sync.drain()

    nc.all_engine_barrier()
    assert self.sems is not None
    popped = nc._tile_sem_poison_stack.pop()
    assert popped is self._sem_poison
    nc.clear_and_free_semaphores(list(self.sems.allocated().values()))
    nc.all_engine_barrier()


def apply_patch():
    tile_mod.TileContext._drain_and_barrier = _patched_drain_and_barrier


def _legalize_waits_json(mod: dict) -> dict:
    """Walrus in this env accepts at most ONE sync wait per instruction.
    Split extra waits onto same-engine NoOp carriers inserted just before."""
    n_new = [0]
    for fn in mod.get('functions', []):
        for blk in fn.get('blocks', []):
            insts = blk.get('instructions', [])
            out = []
            for inst in insts:
                si = inst.get('sync_info') or {}
                waits = si.get('on_wait') or []
                if len(waits) > 1:
                    eng = inst.get('engine')
                    for w in waits[:-1]:
                        n_new[0] += 1
                        out.append({
                            'debug': inst.get('debug', 0),
                            'engine': eng, 'ins': [], 'outs': [],
                            'name': 'I-waitfix-%d' % n_new[0],
                            'opcode': 'NoOp',
                            'sync_info': {'on_update': [], 'on_wait': [w]},
                        })
                    si['on_wait'] = [waits[-1]]
                out.append(inst)
            blk['instructions'] = out
    return mod


def apply_json_patch():
    import orjson
    import concourse.bass as bass_mod
    orig = bass_mod.Bass.to_json_bytes
    def to_json_bytes(self):
        raw = orig(self)
        mod = orjson.loads(raw)
        mod = _legalize_waits_json(mod)
        return orjson.dumps(mod)
    bass_mod.Bass.to_json_bytes = to_json_bytes


import math
import numpy as np
import ml_dtypes
import concourse.bass as bass
import concourse.mybir as mybir
from concourse.tile import TileContext
from concourse import bass_utils
from concourse.masks import make_identity
from concourse import library_config
apply_patch(); apply_json_patch()

f32 = mybir.dt.float32
bf16 = mybir.dt.bfloat16
i32 = mybir.dt.int32
AF = mybir.ActivationFunctionType
ALU = mybir.AluOpType
AX = mybir.AxisListType
P = 128
NC = 8
GRP = 4          # blocks per h PSUM group (GRP*128 f32 = one 2KB bank)
DBG = False

bft = ml_dtypes.bfloat16


def host_prep(inputs, n_tiles_per_core):
    SH = n_tiles_per_core * P
    NPAD = SH * NC
    T = n_tiles_per_core
    N = inputs['x'].shape[0]
    src = np.asarray(inputs['edge_index'][0], np.int64)
    dst = np.asarray(inputs['edge_index'][1], np.int64)
    ea = np.asarray(inputs['edge_attr'], np.float32)
    order = np.argsort(dst, kind='stable')
    src, dst, ea = src[order], dst[order], ea[order]
    core_of = dst // SH
    tile_of = (dst % SH) // P

    counts = np.zeros((NC, T), np.int64)
    for c in range(NC):
        m = core_of == c
        tl, cn = np.unique(tile_of[m], return_counts=True)
        counts[c, tl] = cn
    K = np.maximum(1, np.ceil(counts / P).astype(np.int64).max(axis=0))
    offs = np.concatenate([[0], np.cumsum(K)]).astype(np.int64)
    TOT = int(offs[-1])

    src_pm = np.zeros((NC, P, TOT), np.int32)          # global src node id
    dstl_pm = np.zeros((NC, P, TOT), np.int32)         # dst local to core
    nloc_t = np.full((NC, P, TOT), -1.0, np.float32)   # dst local to tile, pad -1
    src_w = np.zeros((NC, P, TOT * 8), np.int16)       # dma_gather 16-p wrap
    dstl_w = np.zeros((NC, P, TOT * 8), np.int16)
    ea_t = np.zeros((NC, 16, TOT * P), bft)
    for c in range(NC):
        m = core_of == c
        s_c, d_c, e_c, t_c = src[m], dst[m], ea[m], tile_of[m]
        for t in range(T):
            mt = t_c == t
            sc, dc, ec = s_c[mt], d_c[mt], e_c[mt]
            dloc_core = (dc - c * SH).astype(np.int64)
            nloc = dloc_core - t * P
            n_e = len(sc)
            for k in range(int(K[t])):
                blk = int(offs[t]) + k
                lo, hi = k * P, min((k + 1) * P, n_e)
                cnt = max(0, hi - lo)
                if cnt > 0:
                    src_pm[c, :cnt, blk] = sc[lo:hi]
                    dstl_pm[c, :cnt, blk] = dloc_core[lo:hi]
                    nloc_t[c, :cnt, blk] = nloc[lo:hi]
                    ea_t[c, :, blk * P: blk * P + cnt] = ec[lo:hi].T.astype(bft)
    for c in range(NC):
        for t in range(T):
            off, Kt = int(offs[t]), int(K[t])
            lin_s = src_pm[c][:, off:off + Kt].T.ravel()     # i = k*128+p
            lin_d = dstl_pm[c][:, off:off + Kt].T.ravel()
            bs = lin_s.reshape(Kt * 8, 16).T.astype(np.int16)
            bd = lin_d.reshape(Kt * 8, 16).T.astype(np.int16)
            for c8 in range(8):      # replicated per Q7 core stripe
                src_w[c][c8 * 16:(c8 + 1) * 16, off * 8:(off + Kt) * 8] = bs
                dstl_w[c][c8 * 16:(c8 + 1) * 16, off * 8:(off + Kt) * 8] = bd
    s4_h = np.zeros((NC, P, TOT * P), bft)
    st_h = np.zeros((NC, P, TOT * P), bft)
    for c in range(NC):
        a = np.zeros((P, TOT, P), np.float32)
        v = nloc_t[c] >= 0
        ii = np.clip(nloc_t[c].astype(np.int64), 0, 127)
        np.put_along_axis(a, ii[..., None],
                          np.where(v, 1.0, 0.0)[..., None], axis=2)
        s4_h[c] = a.reshape(P, TOT * P).astype(bft)
        st_h[c] = np.ascontiguousarray(a.transpose(2, 1, 0)).reshape(
            P, TOT * P).astype(bft)
    # pooling one-hots + counts
    batch = np.asarray(inputs['batch'], np.int64)
    G_ = 64
    bpool = np.zeros((NC, P, T * 64), bft)
    for c in range(NC):
        base = c * SH
        for t in range(T):
            n0 = base + t * P
            n1 = min(n0 + P, N)
            if n1 > n0:
                rows = np.arange(n1 - n0)
                bpool[c, rows, t * 64 + batch[n0:n1]] = 1.0
    cnts = np.bincount(batch, minlength=G_).astype(np.float32)
    invcnt = (1.0 / np.maximum(cnts, 1.0)).reshape(G_, 1)

    # node data
    xf = np.zeros((NPAD, P), np.float32)
    xf[:N] = np.asarray(inputs['x'], np.float32)
    xT = np.ascontiguousarray(xf.T.astype(bft))            # (128, NPAD)
    pf = np.zeros((NPAD, 4), np.float32)
    pf[:N, 0:3] = np.asarray(inputs['pos'], np.float32)
    # tiled pos: (P, nt*4) with [p, 4t:4t+4] = pos[t*128+p]
    NT_FULL = NPAD // P
    pos_tf = np.ascontiguousarray(
        pf.reshape(NT_FULL, P, 4).transpose(1, 0, 2).reshape(P, NT_FULL * 4))
    iotab = np.tile(np.arange(P, dtype=np.float32), (P, 1))
    return dict(SH=SH, NPAD=NPAD, T=T, K=[int(k) for k in K],
                offs=[int(o) for o in offs], TOT=TOT, src_pm=src_pm,
                dstl_pm=dstl_pm, src_w=src_w, dstl_w=dstl_w,
                nloc_t=nloc_t, s4_h=s4_h, st_h=st_h, ea_t=ea_t, bpool=bpool,
                invcnt=invcnt, xT=xT, pos_tf=pos_tf, iotab=iotab)


def host_weights(inputs):
    w = {}
    for L in range(2):
        mw = np.asarray(inputs[f'l{L}_mlp_w'], np.float32)
        w[f'wproj{L}'] = np.concatenate([mw[0:128], mw[128:256]],
                                        axis=1).astype(bft)     # [dst|src]
        w[f'wea{L}'] = np.ascontiguousarray(mw[256:272]).astype(bft)
        w[f'wrrep{L}'] = np.tile(mw[272:273], (P, 1)).astype(bft)
        ew = np.asarray(inputs[f'l{L}_edge_w'], np.float32)
        eb = np.asarray(inputs[f'l{L}_edge_b'], np.float32)
        we1s = np.zeros((65, 64), np.float32)
        we1s[0:64] = ew; we1s[64] = eb
        w[f'we1s{L}'] = we1s.astype(bft)
        cw = np.asarray(inputs[f'l{L}_coord_w'], np.float32)
        w[f'cwrep{L}'] = np.tile(cw[:, 0][None, :], (P, 1)).astype(bft)
        w[f'cb{L}'] = float(np.asarray(inputs[f'l{L}_coord_b'], np.float32)[0])
        n1 = np.asarray(inputs[f'l{L}_node_w1'], np.float32)
        w[f'wn1x{L}'] = np.ascontiguousarray(n1[0:128]).astype(bft)
        w[f'wn1a{L}'] = np.ascontiguousarray(n1[128:192]).astype(bft)
        w[f'nb1_{L}'] = np.asarray(
            inputs[f'l{L}_node_b1'], np.float32).reshape(64, 1)  # bias col
        w[f'wn2b{L}'] = np.concatenate(
            [np.asarray(inputs[f'l{L}_node_w2'], np.float32),
             np.asarray(inputs[f'l{L}_node_b2'], np.float32)[None, :]],
            0).astype(bft)
    w['wo1'] = np.asarray(inputs['out_w1'], np.float32).astype(bft)
    w['wo1b'] = np.asarray(inputs['out_b1'], np.float32).reshape(P, 1)
    w['wo2'] = np.asarray(inputs['out_w2'], np.float32).astype(bft)
    w['wo2b'] = np.asarray(inputs['out_b2'], np.float32).reshape(32, 1)
    return w


# name -> (shape, dtype); cb0/cb1 are python floats, not tensors
WSPEC = dict(wproj0=((P, P), bf16), wproj1=((P, P), bf16),
             wea0=((16, 64), bf16), wea1=((16, 64), bf16),
             wrrep0=((P, 64), bf16), wrrep1=((P, 64), bf16),
             we1s0=((65, 64), bf16), we1s1=((65, 64), bf16),
             cwrep0=((P, 64), bf16), cwrep1=((P, 64), bf16),
             wn1x0=((P, 64), bf16), wn1x1=((P, 64), bf16),
             wn1a0=((64, 64), bf16), wn1a1=((64, 64), bf16),
             nb1_0=((64, 1), f32), nb1_1=((64, 1), f32),
             wn2b0=((65, P), bf16), wn2b1=((65, P), bf16),
             wo1=((P, P), bf16), wo1b=((P, 1), f32),
             wo2=((P, 32), bf16), wo2b=((32, 1), f32))


def _bc_k(ap2d, Kt):
    """(P, C) -> (P, Kt, C) broadcast view (k stride 0)."""
    a = ap2d.rearrange("p (k c) -> p k c", k=1)
    new = [list(d) for d in a.ap]
    new[1] = [0, Kt]
    return bass.AP(a.tensor, a.offset, new)


def _bc_c(ap2d, C):
    """(P, Kt) -> (P, Kt, C) broadcast view (c stride 0)."""
    a = ap2d.rearrange("p (k o) -> p k o", o=1)
    new = [list(d) for d in a.ap]
    new[2] = [0, C]
    return bass.AP(a.tensor, a.offset, new)


def build(st, cb):
    SH, NPAD, T, K, offs, TOT = (st['SH'], st['NPAD'], st['T'], st['K'],
                                 st['offs'], st['TOT'])
    NT_FULL = NPAD // P
    G_, OUT = 64, 32
    KMAX = max(K)

    nc = bass.Bass("TRN2")
    dram = {}
    def din(name, shape, dt=f32):
        dram[name] = nc.dram_tensor(name, shape, dt, kind="ExternalInput")
        return dram[name]

    xT_full = din('xT_full', (P, NPAD), bf16)
    xT_own = din('xT_own', (P, SH), bf16)
    pos_tf_d = din('pos_tf', (P, NT_FULL * 4))
    pos_own_d = din('pos_own', (P, T * 4))
    i16 = mybir.dt.int16
    srcw_d = din('src_w', (P, TOT * 8), i16)
    dstlw_d = din('dstl_w', (P, TOT * 8), i16)
    nloc_d = din('nloc_t', (P, TOT))
    s4_d = din('s4_h', (P, TOT * P), bf16)
    st_d = din('st_h', (P, TOT * P), bf16)
    ea_td = din('ea_t', (16, TOT * P), bf16)
    bpool_d = din('bpool', (P, T * 64), bf16)
    invcnt_d = din('invcnt', (G_, 1))
    iotab_d = din('iotab', (P, P))
    for n, (shp, dt) in WSPEC.items():
        din(n, shp, dt)
    out_ext = nc.dram_tensor('out', (G_, OUT), f32, kind="ExternalOutput")

    ts0 = nc.dram_tensor('ts0', (NPAD, P), bf16, kind='ExternalOutput')
    ts1sh = nc.dram_tensor('ts1sh', (SH, P), bf16)
    ts1 = nc.dram_tensor('ts1', (NPAD, P), bf16, addr_space="Shared")
    xdp = [nc.dram_tensor('xdp0', (SH, P), bf16, kind='ExternalOutput'),
           nc.dram_tensor('xdp1', (SH, P), bf16)]
    if DBG:
        dbg1 = nc.dram_tensor('dbg1', (P, 68 * 4), bf16, kind='ExternalOutput')
        dbg2 = nc.dram_tensor('dbg2', (P, 64 + P + 64), bf16,
                              kind='ExternalOutput')
        dbg3 = nc.dram_tensor('dbg3', (65, 17 * P), bf16, kind='ExternalOutput')
        dbg5 = nc.dram_tensor('dbg5', (P, 64), bf16, kind='ExternalOutput')
        dbg6 = nc.dram_tensor('dbg6', (P, 4), f32, kind='ExternalOutput')
        dbg7 = nc.dram_tensor('dbg7', (P, P), bf16, kind='ExternalOutput')
        dbg8 = nc.dram_tensor('dbg8', (P, 68), bf16, kind='ExternalOutput')
        dbg9 = nc.dram_tensor('dbg9', (64, P), f32, kind='ExternalOutput')
        dbg10 = nc.dram_tensor('dbg10', (P, 17 * 68), bf16,
                               kind='ExternalOutput')
        dbg11 = nc.dram_tensor('dbg11', (P, 17 * P), bf16,
                               kind='ExternalOutput')
        dbg12 = nc.dram_tensor('dbg12', (P, 17 * 64), bf16,
                               kind='ExternalOutput')
        dbg13 = nc.dram_tensor('dbg13', (P, 17 * 68), bf16,
                               kind='ExternalOutput')
        dbg14 = nc.dram_tensor('dbg14', (P, 17), f32, kind='ExternalOutput')
        dbg15 = nc.dram_tensor('dbg15', (P, 17 * 68), bf16,
                               kind='ExternalOutput')
    gs_in = nc.dram_tensor('gs_in', (G_, P), f32)
    gs_out = nc.dram_tensor('gs_out', (G_, P), f32, addr_space="Shared")

    with TileContext(nc) as tc:
        with (tc.tile_pool(name="pers", bufs=1) as pers,
              tc.tile_pool(name="sa", bufs=3) as sa,
              tc.tile_pool(name="sb", bufs=2) as sb,
              tc.tile_pool(name="sg", bufs=2) as sgp,
              tc.tile_pool(name="ph", bufs=2, space="PSUM") as ps_h,
              tc.tile_pool(name="pe1", bufs=1, space="PSUM") as ps_e1,
              tc.tile_pool(name="ppx", bufs=1, space="PSUM") as ps_ppx,
              tc.tile_pool(name="pagg", bufs=1, space="PSUM") as ps_agg,
              tc.tile_pool(name="pnd", bufs=2, space="PSUM") as ps_nd,
              tc.tile_pool(name="ppool", bufs=1, space="PSUM") as ps_pool):

            nc.gpsimd.load_library(library_config.mlp)
            identb = pers.tile([P, P], bf16, name="identb", tag="identb")
            make_identity(nc, identb[:])
            invcnt_t = pers.tile([G_, 1], f32, name="invc", tag="invc")
            nc.sync.dma_start(invcnt_t[:], invcnt_d[:, :])
            W = {}
            for n, (shp, dt) in WSPEC.items():
                W[n] = pers.tile(list(shp), dt, name="w_" + n, tag="w_" + n)
                nc.sync.dma_start(W[n][:], dram[n][:, :])
            xT_a = pers.tile([P, SH], bf16, name="xT_a", tag="xT_a")
            nc.sync.dma_start(xT_a[:], xT_own[:, :])
            xT_b = pers.tile([P, SH], bf16, name="xT_b", tag="xT_b")
            pos_own = pers.tile([P, T * 4], f32, name="pos_own", tag="pos_own")
            nc.sync.dma_start(pos_own[:], pos_own_d[:, :])
            posn_all = pers.tile([P, T * 4], f32, name="posn_all", tag="posn_all")
            pos_tfull = pers.tile([P, NT_FULL * 4], f32, name="pos_tf",
                                  tag="pos_tf")
            nc.sync.dma_start(pos_tfull[:], pos_tf_d[:, :])
            bpool_t = pers.tile([P, T * 64], bf16, name="bpool", tag="bpool")
            nc.sync.dma_start(bpool_t[:], bpool_d[:, :])
            srcw_t = pers.tile([P, TOT * 8], i16, name="srcw", tag="srcw")
            nc.sync.dma_start(srcw_t[:], srcw_d[:, :])
            xdu = [pers.tile([P, T * 64], bf16, name=f"xdu{i}", tag=f"xdu{i}")
                   for i in range(2)]
            posb = [pers.tile([P, T * 4], bf16, name=f"posb{i}",
                              tag=f"posb{i}") for i in range(2)]
            # persistent double-buffered tiles with constant regions
            hTs_b = [pers.tile([65, KMAX * P], bf16, name=f"hTs{i}", tag=f"hTs{i}")
                     for i in range(2)]
            scat_b = [pers.tile([P, KMAX * 68], bf16, name=f"scat{i}",
                                tag=f"scat{i}") for i in range(2)]
            zst_b = [pers.tile([65, P], bf16, name=f"zst{i}", tag=f"zst{i}")
                     for i in range(2)]
            for i in range(2):
                nc.vector.memset(hTs_b[i][64:65, :], 1.0)
                sc3i = scat_b[i][:].rearrange("p (k c) -> p k c", c=68)
                nc.vector.memset(sc3i[:, :, 64:65], 1.0)
                nc.vector.memset(zst_b[i][64:65, :], 1.0)

            # ---------------- stage A: full-N ts0 table ----------------
            for ti in range(NT_FULL):
                xt = sa.tile([P, P], bf16, name="ax", tag="ax")
                nc.sync.dma_start(xt[:], xT_full[:, ti * P:(ti + 1) * P])
                nb_ = ps_nd.tile([P, 512], f32, name="pnd", tag="pnd",
                                 space="PSUM")
                nc.tensor.matmul(out=nb_[:, 0:64], lhsT=xt[:],
                                 rhs=W['wproj0'][:, 64:128],
                                 start=True, stop=True, skip_group_check=True)
                tst = sa.tile([P, 68], bf16, name="tst", tag="tst")
                nc.scalar.activation(out=tst[:, 0:64], in_=nb_[:, 0:64],
                                     func=AF.Copy)
                nc.vector.tensor_scalar_mul(
                    tst[:, 64:68], pos_tfull[:, ti * 4:(ti + 1) * 4], -1.0)
                nc.sync.dma_start(ts0[ti * P:(ti + 1) * P, 0:68], tst[:])

            # ---------------- stage A-own: xdp0 ------------------------
            for t in range(T):
                nb_ = ps_nd.tile([P, 512], f32, name="pnd", tag="pnd",
                                 space="PSUM")
                nc.tensor.matmul(out=nb_[:, 0:64], lhsT=xT_a[:, t * P:(t + 1) * P],
                                 rhs=W['wproj0'][:, 0:64], start=True, stop=True,
                                 skip_group_check=True)
                xds = sa.tile([P, 68], bf16, name="xds", tag="xds")
                nc.scalar.activation(out=xds[:, 0:64], in_=nb_[:, 0:64],
                                     func=AF.Copy)
                nc.vector.tensor_copy(out=xds[:, 64:68],
                                      in_=pos_own[:, t * 4:(t + 1) * 4])
                nc.vector.tensor_copy(out=xdu[0][:, t * 64:(t + 1) * 64],
                                      in_=xds[:, 0:64])
                nc.vector.tensor_copy(out=posb[0][:, t * 4:(t + 1) * 4],
                                      in_=xds[:, 64:68])

            nidx_regs = {}
            def nidx_reg(n):
                if n not in nidx_regs:
                    nidx_regs[n] = nc.gpsimd.to_reg(n)
                return nidx_regs[n]

            # ---------------- edge + node stage, per layer --------------
            def layer(L, ts_dram, xdp_dram, pos_tile_of, xin_T, xout_T):
                sfx = str(L)
                for t in range(T):
                    Kt = K[t]
                    off = offs[t]
                    ng = (Kt + GRP - 1) // GRP
                    buf = (L * T + t) % 2
                    hT_s = hTs_b[buf]
                    scat = scat_b[buf]
                    zst = zst_b[buf]
                    sc3 = scat[:].rearrange("p (k c) -> p k c", c=68)

                    eat = sgp.tile([16, KMAX * P], bf16, name="eat", tag="eat")
                    nc.sync.dma_start(eat[:, 0:Kt * P],
                                        ea_td[:, off * P:(off + Kt) * P])
                    g_src = sgp.tile([P, KMAX * P], bf16, name="gsrc", tag="gsrc")
                    gs3 = g_src[:].rearrange("p (k c) -> p k c", c=P)
                    k0 = 0
                    while k0 < Kt:
                        kn = min(8, Kt - k0)          # <=1024 descs per ring
                        nc.gpsimd.dma_gather(
                            out_ap=gs3[:, k0:k0 + kn, :], in_ap=ts_dram[:, :],
                            idxs_ap=srcw_t[:, (off + k0) * 8:(off + k0 + kn) * 8],
                            num_idxs=kn * P, num_idxs_reg=nidx_reg(kn * P),
                            elem_size=P)
                        k0 += kn
                    st_t = sgp.tile([P, KMAX * P], bf16, name="stt", tag="stt")
                    nc.sync.dma_start(st_t[:, 0:Kt * P],
                                      st_d[:, off * P:(off + Kt) * P])
                    # dst pos per edge via one-hot matmul
                    ppx = ps_ppx.tile([P, KMAX * 4], f32, name="ppx", tag="ppx",
                                      space="PSUM")
                    for k in range(Kt):
                        nc.tensor.matmul(out=ppx[:, k * 4:(k + 1) * 4],
                                         lhsT=st_t[:, k * P:(k + 1) * P],
                                         rhs=posb[L][:, t * 4:(t + 1) * 4],
                                         start=True, stop=True)
                    gdiff = sb.tile([P, KMAX * 3], bf16, name="gdiff",
                                    tag="gdiff")
                    d3v = gdiff[:].rearrange("p (k c) -> p k c", c=3)
                    nc.vector.tensor_tensor(
                        out=d3v[:, 0:Kt, :],
                        in0=ppx[:].rearrange("p (k c) -> p k c", c=4)[:, 0:Kt, 0:3],
                        in1=gs3[:, 0:Kt, 64:67], op=ALU.add)
                    dsq = sb.tile([P, KMAX * 3], f32, name="dsq", tag="dsq")
                    q3 = dsq[:].rearrange("p (k c) -> p k c", c=3)
                    nc.vector.tensor_tensor(out=q3[:, 0:Kt, :],
                                            in0=d3v[:, 0:Kt, :],
                                            in1=d3v[:, 0:Kt, :], op=ALU.mult)
                    radst = sb.tile([P, KMAX], f32, name="rad", tag="rad")
                    nc.vector.tensor_reduce(
                        out=radst[:, 0:Kt].rearrange("p (k o) -> p k o", o=1),
                        in_=q3[:, 0:Kt, :], axis=AX.X, op=ALU.add)
                    # gsum2 = g_src + radial * w_r  (fused per block)
                    gsum2 = sb.tile([P, KMAX * 64], bf16, name="gsum2",
                                    tag="gsum2")
                    for k in range(Kt):
                        nc.vector.scalar_tensor_tensor(
                            out=gsum2[:, k * 64:(k + 1) * 64],
                            in0=W['wrrep' + sfx][:],
                            scalar=radst[:, k:k + 1],
                            in1=gs3[:, k, 0:64],
                            op0=ALU.mult, op1=ALU.add)
                    # scatter one-hot from host
                    s4 = sb.tile([P, KMAX * P], bf16, name="s4", tag="s4")
                    nc.sync.dma_start(s4[:, 0:Kt * P],
                                      s4_d[:, off * P:(off + Kt) * P])
                    # h'-major h: psum groups of GRP blocks
                    for g in range(ng):
                        nb = min(GRP, Kt - g * GRP)
                        hp = ps_h.tile([64, GRP * P], f32, name="hp", tag="hp",
                                       space="PSUM")
                        for j in range(nb):
                            k = g * GRP + j
                            nc.tensor.matmul(
                                out=hp[:, j * P:(j + 1) * P],
                                lhsT=xdu[L][:, t * 64:(t + 1) * 64],
                                rhs=st_t[:, k * P:(k + 1) * P],
                                start=True, stop=False)
                            nc.tensor.matmul(
                                out=hp[:, j * P:(j + 1) * P],
                                lhsT=gsum2[:, k * 64:(k + 1) * 64],
                                rhs=identb[:],
                                start=False, stop=False)
                            nc.tensor.matmul(
                                out=hp[:, j * P:(j + 1) * P],
                                lhsT=W['wea' + sfx][:],
                                rhs=eat[:, k * P:(k + 1) * P],
                                start=False, stop=True)
                        nc.scalar.activation(
                            out=hT_s[0:64, g * GRP * P:(g * GRP + nb) * P],
                            in_=hp[:, 0:nb * P], func=AF.Silu)
                    # e1 = silu(hT @ we1s) edge-major, into scat[:, :, 0:64]
                    for g in range(ng):
                        nb = min(GRP, Kt - g * GRP)
                        ep = ps_e1.tile([P, GRP * 64], f32, name="ep", tag="ep",
                                        space="PSUM")
                        for j in range(nb):
                            k = g * GRP + j
                            nc.tensor.matmul(out=ep[:, j * 64:(j + 1) * 64],
                                             lhsT=hT_s[:, k * P:(k + 1) * P],
                                             rhs=W['we1s' + sfx][:],
                                             start=True, stop=True)
                        e3 = ep[:, 0:nb * 64].rearrange("p (k c) -> p k c", c=64)
                        nc.scalar.activation(
                            out=sc3[:, g * GRP:g * GRP + nb, 0:64],
                            in_=e3[:, :, :], func=AF.Silu)
                    if L == 0:
                        # sgate = silu(e1 @ cw + cb); coord cols
                        e1m = sb.tile([P, KMAX * 64], bf16, name="e1m",
                                      tag="e1m")
                        m3 = e1m[:].rearrange("p (k c) -> p k c", c=64)
                        nc.vector.tensor_tensor(
                            out=m3[:, 0:Kt, :], in0=sc3[:, 0:Kt, 0:64],
                            in1=_bc_k(W['cwrep' + sfx][:], Kt), op=ALU.mult)
                        sgt = sb.tile([P, KMAX], f32, name="sgt", tag="sgt")
                        nc.vector.tensor_reduce(
                            out=sgt[:, 0:Kt].rearrange("p (k o) -> p k o", o=1),
                            in_=m3[:, 0:Kt, :], axis=AX.X, op=ALU.add)
                        sgs = sb.tile([P, KMAX], bf16, name="sgs", tag="sgs")
                        nc.scalar.activation(out=sgs[:, 0:Kt], in_=sgt[:, 0:Kt],
                                             func=AF.Silu, bias=cb[L])
                        nc.vector.tensor_tensor(out=sc3[:, 0:Kt, 65:68],
                                                in0=d3v[:, 0:Kt, :],
                                                in1=_bc_c(sgs[:, 0:Kt], 3),
                                                op=ALU.mult)
                    # scatter-add into node aggregate
                    NAGG = 68 if L == 0 else 65
                    pagg = ps_agg.tile([P, 68], f32, name="pagg", tag="pagg",
                                       space="PSUM")
                    for k in range(Kt):
                        nc.tensor.matmul(out=pagg[:, 0:NAGG],
                                         lhsT=s4[:, k * P:(k + 1) * P],
                                         rhs=scat[:, k * 68:k * 68 + NAGG],
                                         start=(k == 0), stop=(k == Kt - 1))
                    if DBG and L == 0 and t == 0:
                        nc.sync.dma_start(dbg10[:, 0:Kt * 68],
                                          scat[:, 0:Kt * 68])
                        nc.sync.dma_start(dbg11[:, 0:Kt * P], s4[:, 0:Kt * P])
                        nc.sync.dma_start(dbg12[:, 0:Kt * 64],
                                          gsum2[:, 0:Kt * 64])
                        nc.sync.dma_start(dbg13[:, 0:Kt * 68],
                                          gsum[:, 0:Kt * 68])
                        nc.sync.dma_start(dbg14[:, 0:Kt], radst[:, 0:Kt])
                        nc.sync.dma_start(dbg1[:, 0:68], g_src[:, 0:68])
                        nc.sync.dma_start(dbg1[:, 68:136], g_dst[:, 0:68])
                        nc.sync.dma_start(dbg15[:, 0:Kt * 68],
                                          gs3[:, 0:Kt, 0:68])
                        nc.sync.dma_start(dbg1[:, 136:204], gsum[:, 0:68])
                        nc.sync.dma_start(dbg1[:, 204:272], scat[:, 0:68])
                        nc.sync.dma_start(dbg2[:, 0:64], gsum2[:, 0:64])
                        nc.sync.dma_start(dbg2[:, 64:64 + P], s4[:, 0:P])
                        nc.sync.dma_start(dbg2[:, 64 + P:64 + P + 64],
                                          e1m[:, 0:64])
                        nc.sync.dma_start(dbg3[:, :], hT_s[:, 0:17 * P])
                    # ---- node stage ----
                    eagg = sb.tile([P, 64], bf16, name="eagg", tag="eagg")
                    nc.scalar.activation(out=eagg[:], in_=pagg[:, 0:64],
                                         func=AF.Copy)
                    if L == 0:
                        deg1 = sb.tile([P, 1], f32, name="deg", tag="deg")
                        nc.vector.tensor_scalar_max(deg1[:], pagg[:, 64:65], 1.0)
                        inv = sb.tile([P, 1], f32, name="inv", tag="inv")
                        nc.vector.reciprocal(out=inv[:], in_=deg1[:])
                        posn = sb.tile([P, 4], f32, name="posn", tag="posn")
                        nc.vector.tensor_scalar_mul(posn[:, 0:3], pagg[:, 65:68],
                                                    inv[:, 0:1])
                        nc.vector.tensor_tensor(
                            out=posn[:, 0:3], in0=posn[:, 0:3],
                            in1=pos_tile_of[:, t * 4:t * 4 + 3], op=ALU.add)
                        nc.vector.memset(posn[:, 3:4], 0.0)
                    nb_ = ps_nd.tile([P, 512], f32, name="pnd", tag="pnd",
                                     space="PSUM")
                    nc.tensor.matmul(out=nb_[0:64, 0:P], lhsT=eagg[:],
                                     rhs=identb[:], start=True, stop=True,
                                     skip_group_check=True)
                    eaT = sb.tile([64, P], bf16, name="eaT", tag="eaT")
                    nc.scalar.activation(out=eaT[:], in_=nb_[0:64, 0:P],
                                         func=AF.Copy)
                    nc.tensor.matmul(out=nb_[0:64, P:2 * P],
                                     lhsT=W['wn1x' + sfx][:],
                                     rhs=xin_T[:, t * P:(t + 1) * P],
                                     start=True, stop=False,
                                     skip_group_check=True)
                    nc.tensor.matmul(out=nb_[0:64, P:2 * P],
                                     lhsT=W['wn1a' + sfx][:],
                                     rhs=eaT[:], start=False, stop=True,
                                     skip_group_check=True)
                    nc.scalar.activation(out=zst[0:64, :], in_=nb_[0:64, P:2 * P],
                                         func=AF.Silu,
                                         bias=W['nb1_' + sfx][:, 0:1])
                    nc.tensor.matmul(out=nb_[:, 2 * P:3 * P],
                                     lhsT=W['wn2b' + sfx][:],
                                     rhs=zst[:], start=True, stop=True,
                                     skip_group_check=True)
                    nc.scalar.activation(out=xout_T[:, t * P:(t + 1) * P],
                                         in_=nb_[:, 2 * P:3 * P], func=AF.Copy)
                    if DBG and L == 0 and t == 0:
                        nc.sync.dma_start(dbg5[:, :], eagg[:])
                        nc.sync.dma_start(dbg6[:, :], posn[:])
                        nc.sync.dma_start(dbg7[:, :],
                                          xout_T[:, t * P:(t + 1) * P])
                    if L == 0:
                        nc.tensor.matmul(out=nb_[:, 3 * P:3 * P + 64],
                                         lhsT=xout_T[:, t * P:(t + 1) * P],
                                         rhs=W['wproj1'][:, 64:128],
                                         start=True, stop=True,
                                         skip_group_check=True)
                        tst = sb.tile([P, 68], bf16, name="tst1", tag="tst1")
                        nc.scalar.activation(out=tst[:, 0:64],
                                             in_=nb_[:, 3 * P:3 * P + 64],
                                             func=AF.Copy)
                        nc.vector.tensor_scalar_mul(tst[:, 64:68], posn[:], -1.0)
                        nc.sync.dma_start(ts1sh[t * P:(t + 1) * P, 0:68], tst[:])
                        if DBG and t == 0:
                            nc.sync.dma_start(dbg8[:, :], tst[:])
                        nc.tensor.matmul(out=nb_[:, 3 * P + 64:4 * P],
                                         lhsT=xout_T[:, t * P:(t + 1) * P],
                                         rhs=W['wproj1'][:, 0:64],
                                         start=True, stop=True,
                                         skip_group_check=True)
                        xds = sb.tile([P, 68], bf16, name="xds1", tag="xds1")
                        nc.scalar.activation(out=xds[:, 0:64],
                                             in_=nb_[:, 3 * P + 64:4 * P],
                                             func=AF.Copy)
                        nc.vector.tensor_copy(out=xds[:, 64:68], in_=posn[:])
                        nc.vector.tensor_copy(out=xdu[1][:, t * 64:(t + 1) * 64],
                                              in_=xds[:, 0:64])
                        nc.vector.tensor_copy(out=posb[1][:, t * 4:(t + 1) * 4],
                                              in_=xds[:, 64:68])
                    else:
                        nc.tensor.matmul(out=nb_[:, 3 * P:4 * P],
                                         lhsT=xout_T[:, t * P:(t + 1) * P],
                                         rhs=identb[:], start=True, stop=True,
                                         skip_group_check=True)
                        x2n = sb.tile([P, P], bf16, name="x2n", tag="x2n")
                        nc.scalar.activation(out=x2n[:], in_=nb_[:, 3 * P:4 * P],
                                             func=AF.Copy)
                        nc.tensor.matmul(out=ppool_t[:],
                                         lhsT=bpool_t[:, t * 64:(t + 1) * 64],
                                         rhs=x2n[:],
                                         start=(t == 0), stop=(t == T - 1))

            # layer 0
            tc.strict_bb_all_engine_barrier()
            layer(0, ts0, xdp[0], pos_own, xT_a, xT_b)
            # allgather ts1
            tc.strict_bb_all_engine_barrier()
            nc.gpsimd.collective_compute(
                "AllGather", ALU.bypass, replica_groups=[list(range(NC))],
                ins=[ts1sh.ap().opt()], outs=[ts1.ap().opt()])
            tc.strict_bb_all_engine_barrier()
            # layer 1 (+ pooling accumulation)
            ppool_t = ps_pool.tile([G_, P], f32, name="ppool", tag="ppool",
                                   space="PSUM")
            layer(1, ts1, xdp[1], posn_all, xT_b, xT_a)
            # pooling tail
            gss = sb.tile([G_, P], f32, name="gss", tag="gss")
            nc.vector.tensor_copy(out=gss[:], in_=ppool_t[:])
            nc.sync.dma_start(gs_in[:, :], gss[:])
            if DBG:
                nc.sync.dma_start(dbg9[:, :], gss[:])
            tc.strict_bb_all_engine_barrier()
            nc.gpsimd.collective_compute(
                "AllReduce", ALU.add, replica_groups=[list(range(NC))],
                ins=[gs_in.ap().opt()], outs=[gs_out.ap().opt()])
            tc.strict_bb_all_engine_barrier()
            gsr = sb.tile([G_, P], f32, name="gsr", tag="gsr")
            nc.sync.dma_start(gsr[:], gs_out[:, :])
            gm = sb.tile([G_, P], bf16, name="gm", tag="gm")
            nc.vector.tensor_scalar(out=gm[:], in0=gsr[:],
                                    scalar1=invcnt_t[:, 0:1], scalar2=0.0,
                                    op0=ALU.mult, op1=ALU.max)
            tb = ps_nd.tile([P, 512], f32, name="pnd", tag="pnd", space="PSUM")
            nc.tensor.matmul(out=tb[:, 0:G_], lhsT=gm[:],
                             rhs=identb[0:G_, 0:G_],
                             start=True, stop=True, skip_group_check=True)
            gT = sb.tile([P, G_], bf16, name="gT", tag="gT")
            nc.scalar.activation(out=gT[:], in_=tb[:, 0:G_], func=AF.Copy)
            nc.tensor.matmul(out=tb[:, G_:2 * G_], lhsT=W['wo1'][:], rhs=gT[:],
                             start=True, stop=True, skip_group_check=True)
            r1 = sb.tile([P, G_], bf16, name="r1", tag="r1")
            nc.scalar.activation(out=r1[:], in_=tb[:, G_:2 * G_], func=AF.Relu,
                                 bias=W['wo1b'][:, 0:1])
            nc.tensor.matmul(out=tb[0:32, 2 * G_:3 * G_], lhsT=W['wo2'][:],
                             rhs=r1[:], start=True, stop=True,
                             skip_group_check=True)
            o2 = sb.tile([32, G_], bf16, name="o2", tag="o2")
            with nc.allow_low_precision("final 32x64 to bf16 for PE transpose"):
                nc.vector.tensor_scalar_add(o2[:], tb[0:32, 2 * G_:3 * G_],
                                            W['wo2b'][:, 0:1])
            nc.tensor.matmul(out=tb[0:G_, 3 * G_:3 * G_ + 32], lhsT=o2[:],
                             rhs=identb[0:32, 0:32],
                             start=True, stop=True, skip_group_check=True)
            oT = sb.tile([G_, 32], f32, name="oT", tag="oT")
            nc.scalar.activation(out=oT[:], in_=tb[0:G_, 3 * G_:3 * G_ + 32],
                                 func=AF.Copy)
            nc.sync.dma_start(out_ext[:, :], oT[:])

    from concourse.library_overlay import lower_extended_insts
    lower_extended_insts(nc)
    return nc


def run(inputs, n_tiles_per_core, trace=False):
    st = host_prep(inputs, n_tiles_per_core)
    w = host_weights(inputs)
    SH, T = st['SH'], st['T']
    cb = (w['cb0'], w['cb1'])
    nc = build(st, cb)
    wt = {k: v for k, v in w.items() if k in WSPEC}
    in_maps = []
    for c in range(NC):
        m = dict(xT_full=st['xT'],
                 xT_own=np.ascontiguousarray(st['xT'][:, c * SH:(c + 1) * SH]),
                 pos_tf=st['pos_tf'],
                 pos_own=np.ascontiguousarray(
                     st['pos_tf'][:, c * T * 4:(c + 1) * T * 4]),
                 src_w=st['src_w'][c], dstl_w=st['dstl_w'][c],
                 nloc_t=st['nloc_t'][c], s4_h=st['s4_h'][c],
                 st_h=st['st_h'][c],
                 ea_t=st['ea_t'][c],
                 bpool=st['bpool'][c], invcnt=st['invcnt'],
                 iotab=st['iotab'])
        m.update(wt)
        in_maps.append(m)
    res = bass_utils.run_bass_kernel_spmd(nc, in_maps, core_ids=list(range(NC)),
                                          trace=trace)
    return res


def kernel(**inputs):
    n_tiles = math.ceil(inputs['x'].shape[0] / (P * NC))
    res = run(inputs, n_tiles)
    return res.results[0]['out']
